# revision 14
# baseline (speedup 1.0000x reference)
"""Trainium2 Bass kernel for nn_CausalSelfAttention_15178414424258.

GQA sliding-window causal attention (HQ=16, HK=4, D=64, WINDOW=1024) with
value-embedding gating, rope + qk rms-norm, out-projection.

Sharding: tensor-parallel over the 4 kv-head groups x data-parallel over the
2 batches = 8 cores. Each core handles one batch b and one kv group g
(4 q heads, 1 k head, 1 v head), produces a partial out-projection
(its 256 channels of the attention output against the matching w_o columns);
the host sums the 4 partials per batch.

On-core dataflow (all matmuls fp32r; scores kept transposed [t_k, t_q] so
softmax denominators come free via a ones-column appended to V):
  A) qkv = x @ w_qkv_shard^T (+ gate logit col), gate/sigmoid, v += gate*ve,
     rope via [x1|x1]*[c|s] + [x2|x2]*[-s|c], rms-norm via Ln/Exp rsqrt,
     PE-transpose q/k to [d, t].
  B) per 512-query chunk and head-pair: S^T = k^T q in PSUM, exp on ACT
     (scale=1/8 folds the 1/sqrt(D)), mask/zero edge quarters on GPSIMD,
     PV accumulate with ones-augmented V giving aoT + denom row,
     reciprocal + partition-broadcast-DMA + normalize into aoT.
  C) out_partial = aoT^T @ w_oT_shard, DMA to DRAM.
"""
import sys

sys.path.insert(0, "/opt/trn_rl_repo")

from contextlib import ExitStack  # noqa: E402

import numpy as np  # noqa: E402

import concourse.bass as bass  # noqa: E402
import concourse.tile as tile  # noqa: E402
from concourse import bacc, mybir  # noqa: E402
from concourse.bass_utils import run_bass_kernel_spmd  # noqa: E402

F32 = mybir.dt.float32
F32R = mybir.dt.float32r
AF = mybir.ActivationFunctionType
ALU = mybir.AluOpType
AX = mybir.AxisListType

B, T, E = 2, 2048, 1024
HQ, HK, D = 16, 4, 64
WINDOW = 1024
GATE_CH = 12
RMS_EPS = 1e-8
G = HQ // HK          # 4 q heads per kv group
TB = T // 128         # 16 t-blocks
NC_ = 4               # 512-wide query chunks
KT = E // 128         # 8 k-tiles for the qkv matmul

_CACHE = {}


def _active_m(c):
    return range(max(0, 4 * c - 8), 4 * c + 4)


def build_program():
    nc = bacc.Bacc("TRN2", target_bir_lowering=False, debug=False, num_devices=8)

    xT = nc.declare_dram_parameter("xT", [E, T], F32R, isOutput=False)
    wqkvT = nc.declare_dram_parameter("wqkvT", [E, 386], F32R, isOutput=False)
    ve3 = nc.declare_dram_parameter("ve3", [T, D], F32, isOutput=False)
    ropeA = nc.declare_dram_parameter("ropeA", [T, D], F32, isOutput=False)
    ropeB = nc.declare_dram_parameter("ropeB", [T, D], F32, isOutput=False)
    woT = nc.declare_dram_parameter("woT", [G * D, E], F32R, isOutput=False)
    maskC = nc.declare_dram_parameter("maskC", [128, 128], F32, isOutput=False)
    maskW = nc.declare_dram_parameter("maskW", [128, 128], F32, isOutput=False)
    ident = nc.declare_dram_parameter("ident", [128, 128], F32R, isOutput=False)
    out = nc.declare_dram_parameter("out", [T, E], F32, isOutput=True)

    with tile.TileContext(nc) as tc, ExitStack() as ctx:
        P = lambda **kw: ctx.enter_context(tc.tile_pool(**kw))
        pers = P(name="pers", bufs=1)
        xp = P(name="xp", bufs=2)
        tmp = P(name="tmp", bufs=2)
        p2p = P(name="p2p", bufs=3)
        outs = P(name="outs", bufs=3)
        psb = P(name="psb", bufs=2, space="PSUM")   # [128,1024] scores
        psm = P(name="psm", bufs=2, space="PSUM")   # [128,512] misc
        psa = P(name="psa", bufs=1, space="PSUM")   # [128,1024] PV accum

        # ---- persistent SBUF ----
        wq_sb = [pers.tile([128, 386], F32R, tag=f"wq{k}", name=f"wq{k}") for k in range(KT)]
        wo_sb = [pers.tile([128, E], F32R, tag=f"wo{k}", name=f"wo{k}") for k in range(2)]
        ra_sb = pers.tile([128, TB, D], F32, tag="ra")
        rb_sb = pers.tile([128, TB, D], F32, tag="rb")
        ve_sb = pers.tile([128, TB, D], F32, tag="ve")
        mc_sb = pers.tile([128, 128], F32, tag="mc")
        mw_sb = pers.tile([128, 128], F32, tag="mw")
        v1a = pers.tile([128, TB, 128], F32R, tag="v1a")   # [v | 1 | 0...]
        v1b = pers.tile([128, TB, 128], F32R, tag="v1b")   # [0...| 1 | v]
        qt_sb = [pers.tile([128, T], F32R, tag=f"qt{p}", name=f"qt{p}") for p in range(2)]
        kt_sb = pers.tile([128, T], F32R, tag="kt")  # kT duplicated in both halves
        aot = [pers.tile([128, T], F32R, tag=f"aot{p}", name=f"aot{p}") for p in range(2)]

        wq_r = wqkvT.rearrange("(k p) f -> k p f", p=128)
        wo_r = woT.rearrange("(k p) f -> k p f", p=128)
        for k in range(KT):
            nc.sync.dma_start(wq_sb[k][:], wq_r[k])
        for k in range(2):
            nc.sync.dma_start(wo_sb[k][:], wo_r[k])
        nc.sync.dma_start(ra_sb[:], ropeA.rearrange("(tb p) d -> p tb d", p=128))
        nc.sync.dma_start(rb_sb[:], ropeB.rearrange("(tb p) d -> p tb d", p=128))
        nc.sync.dma_start(ve_sb[:], ve3.rearrange("(tb p) d -> p tb d", p=128))
        nc.sync.dma_start(mc_sb[:], maskC[:])
        nc.sync.dma_start(mw_sb[:], maskW[:])

        # ones/zeros pattern of the augmented V copies
        nc.vector.memset(v1a[:].bitcast(F32), 0.0)
        nc.vector.memset(v1b[:].bitcast(F32), 0.0)
        for tb in range(TB):
            nc.vector.memset(v1a[:, tb, 64:65].bitcast(F32), 1.0)
            nc.vector.memset(v1b[:, tb, 63:64].bitcast(F32), 1.0)

        identity = pers.tile([128, 128], F32R, tag="ident")
        nc.sync.dma_start(identity[:], ident[:])

        xT_r = xT.rearrange("(k p) t -> k p t", p=128)

        # ================= Phase A =================
        for tb in range(TB):
            c, r = divmod(tb, 4)
            if r == 0:
                x_sb = [xp.tile([128, 512], F32R, tag=f"x{k}", name=f"x{k}") for k in range(KT)]
                for k in range(KT):
                    nc.sync.dma_start(x_sb[k][:], xT_r[k, :, c * 512:(c + 1) * 512])
            qkv = psm.tile([128, 512], F32, tag="m")
            for k in range(KT):
                nc.tensor.matmul(qkv[:, 0:386], x_sb[k][:, r * 128:(r + 1) * 128],
                                 wq_sb[k][:], start=(k == 0), stop=(k == KT - 1))

            # gate = sigmoid(logit) via 1/(1+exp(-x)); v = qkv_v + gate*ve3
            eg = tmp.tile([128, 1], F32, tag="eg")
            nc.scalar.activation(eg[:], qkv[:, 384:385], AF.Exp, scale=-1.0)
            gp = tmp.tile([128, 1], F32, tag="gp")
            nc.vector.tensor_scalar_add(gp[:], eg[:], 1.0)
            gi = tmp.tile([128, 1], F32, tag="gi")
            nc.vector.reciprocal_approx_fast(gi[:], gp[:])
            vt = tmp.tile([128, D], F32, tag="vt")
            nc.vector.tensor_scalar_mul(vt[:], ve_sb[:, tb], gi[:])
            nc.vector.tensor_add(v1a[:, tb, 0:64], qkv[:, 320:384], vt[:])
            nc.gpsimd.tensor_copy(v1b[:, tb, 64:128], v1a[:, tb, 0:64])

            # rope: out = [x1|x1]*[c|s] + [x2|x2]*[-s|c]
            def rope(dst, src_ap, nh):
                x1 = src_ap[:, :, 0:32].unsqueeze(2).broadcast_to([128, nh, 2, 32])
                x2 = src_ap[:, :, 32:64].unsqueeze(2).broadcast_to([128, nh, 2, 32])
                rav = (ra_sb[:, tb].rearrange("p (two d) -> p two d", two=2)
                       .unsqueeze(1).broadcast_to([128, nh, 2, 32]))
                rbv = (rb_sb[:, tb].rearrange("p (two d) -> p two d", two=2)
                       .unsqueeze(1).broadcast_to([128, nh, 2, 32]))
                dv = dst[:].rearrange("p (h two d) -> p h two d", h=nh, two=2)
                t1 = tmp.tile([128, nh * 64], F32, tag=f"t1{nh}")
                t1v = t1[:].rearrange("p (h two d) -> p h two d", h=nh, two=2)
                nc.vector.tensor_tensor(t1v, x1, rav, ALU.mult)
                nc.vector.tensor_tensor(dv, x2, rbv, ALU.mult)
                nc.vector.tensor_add(dst[:], dst[:], t1[:])

            qr = tmp.tile([128, G * D], F32, tag="qr")
            rope(qr, qkv[:, 0:256].rearrange("p (h d) -> p h d", h=G), G)
            kr = tmp.tile([128, D], F32, tag="kr")
            rope(kr, qkv[:, 256:320].rearrange("p (h d) -> p h d", h=1), 1)

            # rms-norm scales: rsqrt(mean(x^2)+eps) = exp(-0.5*ln(m))
            sq = tmp.tile([128, G * D], F32, tag="sq")
            nc.vector.tensor_mul(sq[:], qr[:], qr[:])
            sk = tmp.tile([128, D], F32, tag="sk")
            nc.vector.tensor_mul(sk[:], kr[:], kr[:])
            ss = tmp.tile([128, 8], F32, tag="ss")
            nc.vector.tensor_reduce(
                ss[:, 0:4], sq[:].rearrange("p (h d) -> p h d", h=G), AX.X, ALU.add)
            nc.vector.tensor_reduce(
                ss[:, 4:5], sk[:].rearrange("p (h d) -> p h d", h=1), AX.X, ALU.add)
            m5 = tmp.tile([128, 5], F32, tag="m5")
            nc.vector.tensor_scalar(m5[:], ss[:, 0:5], 1.0 / D, RMS_EPS,
                                    ALU.mult, ALU.add)
            ln5 = tmp.tile([128, 5], F32, tag="ln5")
            nc.scalar.activation(ln5[:], m5[:], AF.Ln)
            rs5 = tmp.tile([128, 5], F32, tag="rs5")
            nc.scalar.activation(rs5[:], ln5[:], AF.Exp, scale=-0.5)

            qn = tmp.tile([128, G * D], F32R, tag="qn")
            for h in range(G):
                nc.vector.tensor_scalar_mul(
                    qn[:, h * 64:(h + 1) * 64], qr[:, h * 64:(h + 1) * 64],
                    rs5[:, h:h + 1])
            kn = tmp.tile([128, D], F32R, tag="kn")
            nc.vector.tensor_scalar_mul(kn[:], kr[:], rs5[:, 4:5])

            # transposes to [d, t]
            for p in range(2):
                tq = psm.tile([128, 128], F32R, tag="m")
                nc.tensor.transpose(tq[:], qn[:, p * 128:(p + 1) * 128], identity[:])
                nc.vector.tensor_copy(qt_sb[p][:, tb * 128:(tb + 1) * 128], tq[:])
            tk = psm.tile([64, 128], F32R, tag="m")
            nc.tensor.transpose(tk[:], kn[:], identity[:])
            nc.vector.tensor_copy(kt_sb[0:64, tb * 128:(tb + 1) * 128], tk[:])
            nc.sync.dma_start(kt_sb[64:128, tb * 128:(tb + 1) * 128],
                              kt_sb[0:64, tb * 128:(tb + 1) * 128])

        # ================= Phase B =================
        for c in range(NC_):
            for hp in range(2):
                ms = list(_active_m(c))
                pv = psa.tile([128, 1024], F32, tag="a")
                pva = pv[:, 0:512]
                pvb = pv[:, 512:1024]
                for mi, m in enumerate(ms):
                    s2 = psb.tile([128, 1024], F32, tag="b")
                    for hl in range(2):
                        nc.tensor.matmul(
                            s2[:, hl * 512:(hl + 1) * 512],
                            kt_sb[hl * 64:(hl + 1) * 64,
                                  m * 128:(m + 1) * 128],
                            qt_sb[hp][hl * 64:(hl + 1) * 64,
                                      c * 512:(c + 1) * 512],
                            start=True, stop=True)
                    p2 = p2p.tile([128, 1024], F32R)
                    nc.scalar.activation(p2[:], s2[:], AF.Exp, scale=0.125)
                    # mask edge quarters: delta = i_block - m
                    for qpos in range(4):
                        delta = 4 * c + qpos - m
                        view = p2[:].rearrange("p (h q d) -> p h q d", h=2, q=4)
                        quar = view[:, :, qpos]
                        if delta == 0:
                            mv = mc_sb[:].unsqueeze(1).broadcast_to([128, 2, 128])
                            nc.gpsimd.tensor_tensor(quar, quar, mv, ALU.mult)
                        elif delta == 8:
                            mv = mw_sb[:].unsqueeze(1).broadcast_to([128, 2, 128])
                            nc.gpsimd.tensor_tensor(quar, quar, mv, ALU.mult)
                        elif delta < 0 or delta > 8:
                            for hl in range(2):
                                nc.gpsimd.memset(
                                    p2[:, hl * 512 + qpos * 128:
                                       hl * 512 + (qpos + 1) * 128].bitcast(F32),
                                    0.0)
                    st, sp_ = (mi == 0), (mi == len(ms) - 1)
                    nc.tensor.matmul(pva[:], v1a[:, m], p2[:, 0:512],
                                     start=st, stop=sp_)
                    nc.tensor.matmul(pvb[:], v1b[:, m], p2[:, 512:1024],
                                     start=st, stop=sp_)
                # denominators: reciprocal straight from PSUM rows 63/64,
                # then partition-broadcast via DMA
                ri = tmp.tile([128, 1024], F32, tag="ri")
                nc.vector.reciprocal_approx_fast(ri[:], pv[:, :])
                rb2 = outs.tile([128, 512], F32, tag="rb2")
                nc.sync.dma_start(
                    rb2[0:64, :],
                    ri[64:65, 0:512].unsqueeze(1).broadcast_to([1, 64, 512]))
                nc.sync.dma_start(
                    rb2[64:128, :],
                    ri[63:64, 512:1024].unsqueeze(1).broadcast_to([1, 64, 512]))
                nc.vector.tensor_tensor(
                    aot[hp][0:64, c * 512:(c + 1) * 512],
                    pva[0:64, :], rb2[0:64, :], ALU.mult)
                nc.vector.tensor_tensor(
                    aot[hp][64:128, c * 512:(c + 1) * 512],
                    pvb[64:128, :], rb2[64:128, :], ALU.mult)

        # ================= Phase C =================
        for tb in range(TB):
            for fc in range(2):
                op = psm.tile([128, 512], F32, tag="m")
                for k in range(2):
                    nc.tensor.matmul(op[:],
                                     aot[k][:, tb * 128:(tb + 1) * 128],
                                     wo_sb[k][:, fc * 512:(fc + 1) * 512],
                                     start=(k == 0), stop=(k == 1))
                ob = outs.tile([128, 512], F32, tag="ob")
                if fc == 0:
                    nc.vector.tensor_copy(ob[:], op[:])
                else:
                    nc.scalar.copy(ob[:], op[:])
                nc.sync.dma_start(
                    out[tb * 128:(tb + 1) * 128, fc * 512:(fc + 1) * 512], ob[:])

    nc.compile()
    return nc


def _prep_inputs(x, value_embeds, rope_cos, rope_sin, w_qkv, w_gate, w_o):
    cos = np.asarray(rope_cos, np.float32)
    sin = np.asarray(rope_sin, np.float32)
    ropeA = np.concatenate([cos, sin], axis=1)
    ropeB = np.concatenate([-sin, cos], axis=1)
    ii = np.arange(128)[:, None]
    jj = np.arange(128)[None, :]
    maskC = (ii <= jj).astype(np.float32)   # partition=t_k, free=t_q
    maskW = (ii >= jj).astype(np.float32)
    maps = []
    for core in range(8):
        b, g = divmod(core, 4)
        wq = w_qkv[g * G * D:(g + 1) * G * D]              # [256, E]
        wk = w_qkv[(HQ + g) * D:(HQ + g + 1) * D]          # [64, E]
        wv = w_qkv[(HQ + HK + g) * D:(HQ + HK + g + 1) * D]
        gate_col = np.zeros((2, E), np.float32)
        gate_col[0, :GATE_CH] = w_gate[g]
        wqkvT = np.ascontiguousarray(
            np.concatenate([wq, wk, wv, gate_col], axis=0).T)  # [E, 386]
        maps.append({
            "xT": np.ascontiguousarray(x[b].T),
            "wqkvT": wqkvT,
            "ve3": np.ascontiguousarray(
                3.0 * value_embeds[b, :, g * D:(g + 1) * D]),
            "ropeA": ropeA, "ropeB": ropeB,
            "woT": np.ascontiguousarray(w_o[:, g * G * D:(g + 1) * G * D].T),
            "maskC": maskC, "maskW": maskW,
            "ident": np.eye(128, dtype=np.float32),
        })
    return maps


def kernel(x, value_embeds, rope_cos, rope_sin, w_qkv, w_gate, w_o,
           trace=False):
    if "nc" not in _CACHE:
        _CACHE["nc"] = build_program()
    nc = _CACHE["nc"]
    in_maps = _prep_inputs(x, value_embeds, rope_cos, rope_sin,
                           w_qkv, w_gate, w_o)
    res = run_bass_kernel_spmd(nc, in_maps, list(range(8)), trace=trace)
    _CACHE["last_exec_time_ns"] = res.exec_time_ns
    out = np.empty((B, T, E), np.float32)
    for b in range(B):
        out[b] = sum(res.results[4 * b + g]["out"] for g in range(4))
    return out


# revision 24
# speedup vs baseline: 482.6462x; 482.6462x over previous
"""Trainium2 Bass kernel for nn_CausalSelfAttention_15178414424258.

GQA sliding-window causal attention (HQ=16, HK=4, D=64, WINDOW=1024) with
value-embedding gating, rope + qk rms-norm, out-projection.

Sharding: tensor-parallel over the 4 kv-head groups x data-parallel over the
2 batches = 8 cores. Each core handles one batch b and one kv group g
(4 q heads, 1 k head, 1 v head), produces a partial out-projection
(its 256 channels of the attention output against the matching w_o columns);
the host sums the 4 partials per batch.

On-core dataflow (all matmuls fp32r; scores kept transposed [t_k, t_q] so
softmax denominators come free via a ones-column appended to V):
  A) qkv = x @ w_qkv_shard^T (+ gate logit col), gate/sigmoid, v += gate*ve,
     rope via [x1|x1]*[c|s] + [x2|x2]*[-s|c], rms-norm via Ln/Exp rsqrt,
     PE-transpose q/k to [d, t].
  B) per 512-query chunk and head-pair: S^T = k^T q in PSUM, exp on ACT
     (scale=1/8 folds the 1/sqrt(D)), mask/zero edge quarters on GPSIMD,
     PV accumulate with ones-augmented V giving aoT + denom row,
     reciprocal + partition-broadcast-DMA + normalize into aoT.
  C) out_partial = aoT^T @ w_oT_shard, DMA to DRAM.
"""
import sys

sys.path.insert(0, "/opt/trn_rl_repo")

from contextlib import ExitStack  # noqa: E402

import numpy as np  # noqa: E402

import concourse.bass as bass  # noqa: E402
import concourse.tile as tile  # noqa: E402
from concourse import bacc, mybir  # noqa: E402
from concourse.bass_utils import run_bass_kernel_spmd  # noqa: E402

F32 = mybir.dt.float32
F32R = mybir.dt.float32r
BF16 = mybir.dt.bfloat16
AF = mybir.ActivationFunctionType
ALU = mybir.AluOpType
AX = mybir.AxisListType

B, T, E = 2, 2048, 1024
HQ, HK, D = 16, 4, 64
WINDOW = 1024
GATE_CH = 12
RMS_EPS = 1e-8
G = HQ // HK          # 4 q heads per kv group
TB = T // 128         # 16 t-blocks
NC_ = 4               # 512-wide query chunks
KT = E // 128         # 8 k-tiles for the qkv matmul

_CACHE = {}


def _active_m(c):
    return range(max(0, 4 * c - 8), 4 * c + 4)


def _pin_act_tables(nc):
    """Keep Exp/Ln only in the combined set so insert_act_table_loads
    emits a single table load instead of thrashing between sets."""
    from concourse import hw_specs
    tabs = hw_specs.get_activation_tables(nc.m.arch)
    for name, s in tabs.items():
        if name != "natural_log_exp_and_others":
            s.discard(AF.Exp)
            s.discard(AF.Ln)


def build_program():
    nc = bacc.Bacc("TRN2", target_bir_lowering=False, debug=False, num_devices=8)
    _pin_act_tables(nc)

    xT = nc.declare_dram_parameter("xT", [E, T], F32R, isOutput=False)
    wqkvT = nc.declare_dram_parameter("wqkvT", [E, 386], F32R, isOutput=False)
    ve3 = nc.declare_dram_parameter("ve3", [T, D], F32, isOutput=False)
    ropeA = nc.declare_dram_parameter("ropeA", [T, D], F32, isOutput=False)
    ropeB = nc.declare_dram_parameter("ropeB", [T, D], F32, isOutput=False)
    woT = nc.declare_dram_parameter("woT", [G * D, E], F32R, isOutput=False)
    maskC = nc.declare_dram_parameter("maskC", [128, 128], BF16, isOutput=False)
    maskW = nc.declare_dram_parameter("maskW", [128, 128], BF16, isOutput=False)
    ident = nc.declare_dram_parameter("ident", [128, 128], F32R, isOutput=False)
    identb = nc.declare_dram_parameter("identb", [128, 128], BF16, isOutput=False)
    out = nc.declare_dram_parameter("out", [T, E], F32, isOutput=True)

    with tile.TileContext(nc) as tc, ExitStack() as ctx:
        P = lambda **kw: ctx.enter_context(tc.tile_pool(**kw))
        pers = P(name="pers", bufs=1)
        xp = P(name="xp", bufs=2)
        tmp = P(name="tmp", bufs=2)
        p2p = P(name="p2p", bufs=4)
        outs = P(name="outs", bufs=3)
        # PSUM budget (8 banks): tag "s" 2x[128,1024] (scores + qkv/outproj),
        # tags "a0"/"a1" 1x[128,1024] each (PV accumulators, hp-alternating,
        # reused for the phase-A transposes)
        ps = P(name="ps", bufs=1, space="PSUM")

        # ---- persistent SBUF ----
        wq_sb = [pers.tile([128, 386], F32R, tag=f"wq{k}", name=f"wq{k}") for k in range(KT)]
        wo_sb = [pers.tile([128, E], F32R, tag=f"wo{k}", name=f"wo{k}") for k in range(2)]
        ra_sb = pers.tile([128, TB, D], F32, tag="ra")
        rb_sb = pers.tile([128, TB, D], F32, tag="rb")
        ve_sb = pers.tile([128, TB, D], F32, tag="ve")
        mc_sb = pers.tile([128, 128], BF16, tag="mc")
        mw_sb = pers.tile([128, 128], BF16, tag="mw")
        v1a = pers.tile([128, TB, 128], F32R, tag="v1a")   # [v | 1 | 0...]
        v1b = pers.tile([128, TB, 128], F32R, tag="v1b")   # [0...| 1 | v]
        qt_sb = [pers.tile([128, T], F32R, tag=f"qt{p}", name=f"qt{p}") for p in range(2)]
        kt_sb = pers.tile([128, T], F32R, tag="kt")  # kT duplicated in both halves
        aot = [pers.tile([128, T], F32R, tag=f"aot{p}", name=f"aot{p}") for p in range(2)]

        wq_r = wqkvT.rearrange("(k p) f -> k p f", p=128)
        wo_r = woT.rearrange("(k p) f -> k p f", p=128)
        xT_r0 = xT.rearrange("(k p) t -> k p t", p=128)
        x_first = [xp.tile([128, 512], F32R, tag=f"x{k}", name=f"x{k}")
                   for k in range(KT)]
        for k in range(KT):
            nc.sync.dma_start(x_first[k][:], xT_r0[k, :, 0:512])
            nc.sync.dma_start(wq_sb[k][:], wq_r[k])
        nc.sync.dma_start(ra_sb[:], ropeA.rearrange("(tb p) d -> p tb d", p=128))
        nc.sync.dma_start(rb_sb[:], ropeB.rearrange("(tb p) d -> p tb d", p=128))
        nc.sync.dma_start(ve_sb[:], ve3.rearrange("(tb p) d -> p tb d", p=128))
        for k in range(2):
            nc.sync.dma_start(wo_sb[k][:], wo_r[k])
        nc.sync.dma_start(mc_sb[:], maskC[:])
        nc.sync.dma_start(mw_sb[:], maskW[:])

        # ones/zeros pattern of the augmented V copies
        nc.vector.memset(v1a[:].bitcast(F32), 0.0)
        nc.vector.memset(v1b[:].bitcast(F32), 0.0)
        for tb in range(TB):
            nc.vector.memset(v1a[:, tb, 64:65].bitcast(F32), 1.0)
            nc.vector.memset(v1b[:, tb, 63:64].bitcast(F32), 1.0)

        identity = pers.tile([128, 128], F32R, tag="ident")
        nc.sync.dma_start(identity[:], ident[:])
        identity_b = pers.tile([128, 128], BF16, tag="identb")
        nc.sync.dma_start(identity_b[:], identb[:])

        xT_r = xT.rearrange("(k p) t -> k p t", p=128)

        # ================= Phase A =================
        qn_kn = {}
        for tb in range(TB):
            c, r = divmod(tb, 4)
            if r == 0:
                if c == 0:
                    x_sb = x_first
                else:
                    x_sb = [xp.tile([128, 512], F32R, tag=f"x{k}",
                                    name=f"x{k}") for k in range(KT)]
                    for k in range(KT):
                        nc.sync.dma_start(x_sb[k][:],
                                          xT_r[k, :, c * 512:(c + 1) * 512])
            qkv_ps = ps.tile([128, 1024], F32, tag="s", name="qkv_ps",
                             bufs=2)[:, 0:512]
            for k in range(KT):
                nc.tensor.matmul(qkv_ps[:, 0:386],
                                 x_sb[k][:, r * 128:(r + 1) * 128],
                                 wq_sb[k][:], start=(k == 0), stop=(k == KT - 1))
            # PSUM -> SBUF once (ACT) so rope/v-gate can run on GPSIMD
            qkv = tmp.tile([128, 386], F32, tag="qkvs")
            nc.scalar.copy(qkv[:], qkv_ps[:, 0:386])

            # gate = sigmoid(logit) via 1/(1+exp(-x)); v = qkv_v + gate*ve3
            eg = tmp.tile([128, 1], F32, tag="eg")
            nc.scalar.activation(eg[:], qkv[:, 384:385], AF.Exp, scale=-1.0)
            gp = tmp.tile([128, 1], F32, tag="gp")
            nc.vector.tensor_scalar_add(gp[:], eg[:], 1.0)
            gi = tmp.tile([128, 1], F32, tag="gi")
            nc.vector.reciprocal_approx_fast(gi[:], gp[:])
            vt = tmp.tile([128, D], F32, tag="vt")
            nc.vector.tensor_scalar_mul(vt[:], ve_sb[:, tb], gi[:])
            nc.vector.tensor_add(v1a[:, tb, 0:64], qkv[:, 320:384], vt[:])
            nc.gpsimd.tensor_copy(v1b[:, tb, 64:128], v1a[:, tb, 0:64])

            # rope: out = [x1|x1]*[c|s] + [x2|x2]*[-s|c]
            def rope(dst, src_ap, nh, eng):
                x1 = src_ap[:, :, 0:32].unsqueeze(2).broadcast_to([128, nh, 2, 32])
                x2 = src_ap[:, :, 32:64].unsqueeze(2).broadcast_to([128, nh, 2, 32])
                rav = (ra_sb[:, tb].rearrange("p (two d) -> p two d", two=2)
                       .unsqueeze(1).broadcast_to([128, nh, 2, 32]))
                rbv = (rb_sb[:, tb].rearrange("p (two d) -> p two d", two=2)
                       .unsqueeze(1).broadcast_to([128, nh, 2, 32]))
                dv = dst[:].rearrange("p (h two d) -> p h two d", h=nh, two=2)
                t1 = tmp.tile([128, nh * 64], F32, tag=f"t1{nh}")
                t1v = t1[:].rearrange("p (h two d) -> p h two d", h=nh, two=2)
                eng.tensor_tensor(t1v, x1, rav, ALU.mult)
                eng.tensor_tensor(dv, x2, rbv, ALU.mult)
                eng.tensor_add(dst[:], dst[:], t1[:])

            qr = tmp.tile([128, G * D], F32, tag="qr")
            rope(qr, qkv[:, 0:256].rearrange("p (h d) -> p h d", h=G), G,
                 nc.gpsimd)
            kr = tmp.tile([128, D], F32, tag="kr")
            rope(kr, qkv[:, 256:320].rearrange("p (h d) -> p h d", h=1), 1,
                 nc.vector)

            # rms-norm scales: rsqrt(mean(x^2)+eps) = exp(-0.5*ln(m))
            sq = tmp.tile([128, D], F32, tag="sq")
            ss = tmp.tile([128, 8], F32, tag="ss")
            for h in range(G):
                nc.scalar.activation(sq[:], qr[:, h * 64:(h + 1) * 64],
                                     AF.Square, accum_out=ss[:, h:h + 1])
            nc.scalar.activation(sq[:], kr[:], AF.Square,
                                 accum_out=ss[:, 4:5])
            m5 = tmp.tile([128, 5], F32, tag="m5")
            nc.vector.tensor_scalar(m5[:], ss[:, 0:5], 1.0 / D, RMS_EPS,
                                    ALU.mult, ALU.add)
            ln5 = tmp.tile([128, 5], F32, tag="ln5")
            nc.scalar.activation(ln5[:], m5[:], AF.Ln)
            rs5 = tmp.tile([128, 5], F32, tag="rs5")
            nc.scalar.activation(rs5[:], ln5[:], AF.Exp, scale=-0.5)

            qn = tmp.tile([128, G * D], F32R, tag="qn", bufs=4)
            for h in range(G):
                nc.vector.tensor_scalar_mul(
                    qn[:, h * 64:(h + 1) * 64], qr[:, h * 64:(h + 1) * 64],
                    rs5[:, h:h + 1])
            kn = tmp.tile([128, D], F32R, tag="kn", bufs=4)
            nc.vector.tensor_scalar_mul(kn[:], kr[:], rs5[:, 4:5])

            # transposes run 2 iterations behind so PE never waits on the
            # rope/rms chain of the current block
            qn_kn[tb] = (qn, kn)
            for dtb in ([tb - 2] if tb >= 2 else []) + \
                       ([tb - 1, tb] if tb == TB - 1 else []):
                dqn, dkn = qn_kn.pop(dtb)
                for p in range(2):
                    tq = ps.tile([128, 1024], F32R, tag=("a0", "a1")[p],
                                 name="tq", bufs=1)[:, 0:128]
                    nc.tensor.transpose(tq[:], dqn[:, p * 128:(p + 1) * 128],
                                        identity[:])
                    nc.vector.tensor_copy(
                        qt_sb[p][:, dtb * 128:(dtb + 1) * 128], tq[:])
                tk = ps.tile([128, 1024], F32R, tag="a0",
                             name="tk", bufs=1)[0:64, 0:128]
                nc.tensor.transpose(tk[:], dkn[:], identity[:])
                nc.vector.tensor_copy(kt_sb[0:64, dtb * 128:(dtb + 1) * 128],
                                      tk[:])
                nc.sync.dma_start(kt_sb[64:128, dtb * 128:(dtb + 1) * 128],
                                  kt_sb[0:64, dtb * 128:(dtb + 1) * 128])

        # ========== Phase B + C, interleaved per 512-query chunk ==========
        # Both head-pair streams advance m-by-m in lockstep so the ACT
        # engine (exp) stays saturated; out-projection for the finished
        # chunk is emitted immediately so its PSUM->SBUF copies and output
        # DMAs overlap the next chunk's attention.
        for c in range(NC_):
            ms = list(_active_m(c))
            pvs = [ps.tile([128, 1024], F32, tag=("a0", "a1")[hp],
                           name="pv", bufs=1) for hp in range(2)]
            # order blocks so a full-span m comes first: its PV matmul
            # (start=True) initializes the whole accumulator, letting every
            # later PV run trimmed to its active span without memsets.
            spans = {}
            for m in ms:
                deltas = [4 * c + qpos - m for qpos in range(4)]
                act_q = [q for q in range(4) if 0 <= deltas[q] <= 8]
                spans[m] = (act_q[0], act_q[-1] + 1, deltas)
            mf = next(m for m in ms if spans[m][0] == 0 and spans[m][1] == 4)
            ms_o = [mf] + [m for m in ms if m != mf]
            pending = {}   # hp -> (p2, mi) awaiting its PV matmuls
            for mi in range(len(ms_o) + 1):
                for hp in range(2):
                    if mi < len(ms_o):
                        m = ms_o[mi]
                        qs, qe, deltas = spans[m]
                        sqs, sqe = qs, qe
                        if sqe - sqs == 1:           # N=128 runs at 1/4 rate;
                            if sqs >= 1:             # widen to 256 (even, fast)
                                sqs -= 1
                            else:
                                sqe += 1
                        w = (sqe - sqs) * 128
                        s2 = ps.tile([128, 1024], F32, tag="s", name="s2",
                                     bufs=2)
                        for hl in range(2):
                            o = hl * 512 + sqs * 128
                            nc.tensor.matmul(
                                s2[:, o:o + w],
                                kt_sb[hl * 64:(hl + 1) * 64,
                                      m * 128:(m + 1) * 128],
                                qt_sb[hp][hl * 64:(hl + 1) * 64,
                                          c * 512 + sqs * 128:
                                          c * 512 + sqe * 128],
                                start=True, stop=False,
                                tile_position=(hl * 64, 0),
                                skip_group_check=True)
                            for qpos in range(qs, qe):
                                mt = (mc_sb if deltas[qpos] == 0 else
                                      mw_sb if deltas[qpos] == 8 else None)
                                if mt is None:
                                    continue
                                qo = hl * 512 + qpos * 128
                                nc.tensor.matmul(
                                    s2[:, qo:qo + 128], identity_b[:], mt[:],
                                    start=False, stop=False,
                                    skip_group_check=True)
                        p2 = p2p.tile([128, 1024], F32R)
                        p2v = p2[:].rearrange("p (h f) -> p h f", h=2)
                        s2v = s2[:].rearrange("p (h f) -> p h f", h=2)
                        nc.scalar.activation(
                            p2v[:, :, qs * 128:qe * 128],
                            s2v[:, :, qs * 128:qe * 128],
                            AF.Exp, scale=0.125)
                    if mi > 0:
                        prev_p2, pmi = pending[hp]
                        pm = ms_o[pmi]
                        pqs, pqe, _ = spans[pm]
                        st = (pmi == 0)
                        sp_ = (pmi == len(ms_o) - 1)
                        if st:
                            pqs, pqe = 0, 4
                        pw = (pqe - pqs) * 128
                        for half in range(2):
                            o = half * 512 + pqs * 128
                            nc.tensor.matmul(
                                pvs[hp][:, o:o + pw],
                                (v1a, v1b)[half][:, pm],
                                prev_p2[:, o:o + pw],
                                start=st, stop=sp_, skip_group_check=True)
                    if mi < len(ms_o):
                        pending[hp] = (p2, mi)
            for hp in range(2):
                pv = pvs[hp]
                # denominators: reciprocal straight from PSUM rows 63/64,
                # then partition-broadcast via DMA
                ri = tmp.tile([128, 1024], F32, tag="ri")
                nc.vector.reciprocal_approx_fast(ri[:], pv[:, :])
                rb2 = outs.tile([128, 512], F32, tag="rb2")
                nc.sync.dma_start(
                    rb2[0:64, :],
                    ri[64:65, 0:512].unsqueeze(1).broadcast_to([1, 64, 512]))
                nc.sync.dma_start(
                    rb2[64:128, :],
                    ri[63:64, 512:1024].unsqueeze(1).broadcast_to([1, 64, 512]))
                nc.vector.tensor_tensor(
                    aot[hp][0:64, c * 512:(c + 1) * 512],
                    pv[0:64, 0:512], rb2[0:64, :], ALU.mult)
                nc.vector.tensor_tensor(
                    aot[hp][64:128, c * 512:(c + 1) * 512],
                    pv[64:128, 512:1024], rb2[64:128, :], ALU.mult)
            # out-projection for this chunk
            for tb in range(4 * c, 4 * c + 4):
                for fc in range(2):
                    op = ps.tile([128, 1024], F32, tag=("a1", "a0")[fc],
                                 name="op", bufs=1)[:, 0:512]
                    for k in range(2):
                        nc.tensor.matmul(op[:],
                                         aot[k][:, tb * 128:(tb + 1) * 128],
                                         wo_sb[k][:, fc * 512:(fc + 1) * 512],
                                         start=(k == 0), stop=(k == 1))
                    ob = outs.tile([128, 512], F32, tag="ob")
                    nc.vector.tensor_copy(ob[:], op[:])
                    nc.sync.dma_start(
                        out[tb * 128:(tb + 1) * 128,
                            fc * 512:(fc + 1) * 512], ob[:])

    nc.compile()
    return nc


def _prep_inputs(x, value_embeds, rope_cos, rope_sin, w_qkv, w_gate, w_o):
    cos = np.asarray(rope_cos, np.float32)
    sin = np.asarray(rope_sin, np.float32)
    ropeA = np.concatenate([cos, sin], axis=1)
    ropeB = np.concatenate([-sin, cos], axis=1)
    ii = np.arange(128)[:, None]
    jj = np.arange(128)[None, :]
    import ml_dtypes
    maskC = np.where(ii <= jj, 0.0, -1e30).astype(ml_dtypes.bfloat16)
    maskW = np.where(ii >= jj, 0.0, -1e30).astype(ml_dtypes.bfloat16)
    maps = []
    for core in range(8):
        b, g = divmod(core, 4)
        wq = w_qkv[g * G * D:(g + 1) * G * D]              # [256, E]
        wk = w_qkv[(HQ + g) * D:(HQ + g + 1) * D]          # [64, E]
        wv = w_qkv[(HQ + HK + g) * D:(HQ + HK + g + 1) * D]
        gate_col = np.zeros((2, E), np.float32)
        gate_col[0, :GATE_CH] = w_gate[g]
        wqkvT = np.ascontiguousarray(
            np.concatenate([wq, wk, wv, gate_col], axis=0).T)  # [E, 386]
        maps.append({
            "xT": np.ascontiguousarray(x[b].T),
            "wqkvT": wqkvT,
            "ve3": np.ascontiguousarray(
                3.0 * value_embeds[b, :, g * D:(g + 1) * D]),
            "ropeA": ropeA, "ropeB": ropeB,
            "woT": np.ascontiguousarray(w_o[:, g * G * D:(g + 1) * G * D].T),
            "maskC": maskC, "maskW": maskW,
            "ident": np.eye(128, dtype=np.float32),
            "identb": np.eye(128, dtype=ml_dtypes.bfloat16),
        })
    return maps


def kernel(x, value_embeds, rope_cos, rope_sin, w_qkv, w_gate, w_o,
           trace=False):
    if "nc" not in _CACHE:
        _CACHE["nc"] = build_program()
    nc = _CACHE["nc"]
    in_maps = _prep_inputs(x, value_embeds, rope_cos, rope_sin,
                           w_qkv, w_gate, w_o)
    res = run_bass_kernel_spmd(nc, in_maps, list(range(8)), trace=trace)
    _CACHE["last_exec_time_ns"] = res.exec_time_ns
    out = np.empty((B, T, E), np.float32)
    for b in range(B):
        out[b] = sum(res.results[4 * b + g]["out"] for g in range(4))
    return out


# revision 26
# speedup vs baseline: 506.1721x; 1.0487x over previous
"""Trainium2 Bass kernel for nn_CausalSelfAttention_15178414424258.

GQA sliding-window causal attention (HQ=16, HK=4, D=64, WINDOW=1024) with
value-embedding gating, rope + qk rms-norm, out-projection.

Sharding: tensor-parallel over the 4 kv-head groups x data-parallel over the
2 batches = 8 cores. Each core handles one batch b and one kv group g
(4 q heads, 1 k head, 1 v head), produces a partial out-projection
(its 256 channels of the attention output against the matching w_o columns);
the host sums the 4 partials per batch.

On-core dataflow (all matmuls fp32r; scores kept transposed [t_k, t_q] so
softmax denominators come free via a ones-column appended to V):
  A) qkv = x @ w_qkv_shard^T (+ gate logit col), gate/sigmoid, v += gate*ve,
     rope via [x1|x1]*[c|s] + [x2|x2]*[-s|c], rms-norm via Ln/Exp rsqrt,
     PE-transpose q/k to [d, t].
  B) per 512-query chunk and head-pair: S^T = k^T q in PSUM, exp on ACT
     (scale=1/8 folds the 1/sqrt(D)), mask/zero edge quarters on GPSIMD,
     PV accumulate with ones-augmented V giving aoT + denom row,
     reciprocal + partition-broadcast-DMA + normalize into aoT.
  C) out_partial = aoT^T @ w_oT_shard, DMA to DRAM.
"""
import sys

sys.path.insert(0, "/opt/trn_rl_repo")

from contextlib import ExitStack  # noqa: E402

import numpy as np  # noqa: E402

import concourse.bass as bass  # noqa: E402
import concourse.tile as tile  # noqa: E402
from concourse import bacc, mybir  # noqa: E402
from concourse.bass_utils import run_bass_kernel_spmd  # noqa: E402

F32 = mybir.dt.float32
F32R = mybir.dt.float32r
BF16 = mybir.dt.bfloat16
AF = mybir.ActivationFunctionType
ALU = mybir.AluOpType
AX = mybir.AxisListType

B, T, E = 2, 2048, 1024
HQ, HK, D = 16, 4, 64
WINDOW = 1024
GATE_CH = 12
RMS_EPS = 1e-8
G = HQ // HK          # 4 q heads per kv group
TB = T // 128         # 16 t-blocks
NC_ = 4               # 512-wide query chunks
KT = E // 128         # 8 k-tiles for the qkv matmul

_CACHE = {}


def _active_m(c):
    return range(max(0, 4 * c - 8), 4 * c + 4)


def _pin_act_tables(nc):
    """Keep Exp/Ln only in the combined set so insert_act_table_loads
    emits a single table load instead of thrashing between sets."""
    from concourse import hw_specs
    tabs = hw_specs.get_activation_tables(nc.m.arch)
    for name, s in tabs.items():
        if name != "natural_log_exp_and_others":
            s.discard(AF.Exp)
            s.discard(AF.Ln)


def build_program():
    nc = bacc.Bacc("TRN2", target_bir_lowering=False, debug=False, num_devices=8)
    _pin_act_tables(nc)

    xT = nc.declare_dram_parameter("xT", [E, T], F32R, isOutput=False)
    wqkvT = nc.declare_dram_parameter("wqkvT", [E, 386], F32R, isOutput=False)
    ve3 = nc.declare_dram_parameter("ve3", [T, D], F32, isOutput=False)
    ropeA = nc.declare_dram_parameter("ropeA", [T, D], F32, isOutput=False)
    ropeB = nc.declare_dram_parameter("ropeB", [T, D], F32, isOutput=False)
    woT = nc.declare_dram_parameter("woT", [G * D, E], F32R, isOutput=False)
    maskC = nc.declare_dram_parameter("maskC", [128, 128], BF16, isOutput=False)
    maskW = nc.declare_dram_parameter("maskW", [128, 128], BF16, isOutput=False)
    ident = nc.declare_dram_parameter("ident", [128, 128], F32R, isOutput=False)
    identb = nc.declare_dram_parameter("identb", [128, 128], BF16, isOutput=False)
    out = nc.declare_dram_parameter("out", [T, E], F32, isOutput=True)

    with tile.TileContext(nc) as tc, ExitStack() as ctx:
        P = lambda **kw: ctx.enter_context(tc.tile_pool(**kw))
        pers = P(name="pers", bufs=1)
        xp = P(name="xp", bufs=2)
        tmp = P(name="tmp", bufs=2)
        p2p = P(name="p2p", bufs=4)
        outs = P(name="outs", bufs=3)
        # PSUM budget (8 banks): tag "s" 2x[128,1024] (scores + qkv/outproj),
        # tags "a0"/"a1" 1x[128,1024] each (PV accumulators, hp-alternating,
        # reused for the phase-A transposes)
        ps = P(name="ps", bufs=1, space="PSUM")

        # ---- persistent SBUF ----
        wq_sb = [pers.tile([128, 386], F32R, tag=f"wq{k}", name=f"wq{k}") for k in range(KT)]
        wo_sb = [pers.tile([128, E], F32R, tag=f"wo{k}", name=f"wo{k}") for k in range(2)]
        ra_sb = pers.tile([128, TB, D], F32, tag="ra")
        rb_sb = pers.tile([128, TB, D], F32, tag="rb")
        ve_sb = pers.tile([128, TB, D], F32, tag="ve")
        mc_sb = pers.tile([128, 128], BF16, tag="mc")
        mw_sb = pers.tile([128, 128], BF16, tag="mw")
        v1a = pers.tile([128, TB, 128], F32R, tag="v1a")   # [v | 1 | 0...]
        v1b = pers.tile([128, TB, 128], F32R, tag="v1b")   # [0...| 1 | v]
        qt_sb = [pers.tile([128, T], F32R, tag=f"qt{p}", name=f"qt{p}") for p in range(2)]
        kt_sb = pers.tile([128, T], F32R, tag="kt")  # kT duplicated in both halves
        aot = [pers.tile([128, T], F32R, tag=f"aot{p}", name=f"aot{p}") for p in range(2)]

        wq_r = wqkvT.rearrange("(k p) f -> k p f", p=128)
        wo_r = woT.rearrange("(k p) f -> k p f", p=128)
        xT_r0 = xT.rearrange("(k p) t -> k p t", p=128)
        x_first = [xp.tile([128, 512], F32R, tag=f"x{k}", name=f"x{k}")
                   for k in range(KT)]
        for k in range(KT):
            nc.sync.dma_start(x_first[k][:], xT_r0[k, :, 0:512])
            nc.sync.dma_start(wq_sb[k][:], wq_r[k])
        nc.sync.dma_start(ra_sb[:], ropeA.rearrange("(tb p) d -> p tb d", p=128))
        nc.sync.dma_start(rb_sb[:], ropeB.rearrange("(tb p) d -> p tb d", p=128))
        nc.sync.dma_start(ve_sb[:], ve3.rearrange("(tb p) d -> p tb d", p=128))
        for k in range(2):
            nc.sync.dma_start(wo_sb[k][:], wo_r[k])
        nc.sync.dma_start(mc_sb[:], maskC[:])
        nc.sync.dma_start(mw_sb[:], maskW[:])

        # ones/zeros pattern of the augmented V copies
        nc.vector.memset(v1a[:].bitcast(F32), 0.0)
        nc.vector.memset(v1b[:].bitcast(F32), 0.0)
        for tb in range(TB):
            nc.vector.memset(v1a[:, tb, 64:65].bitcast(F32), 1.0)
            nc.vector.memset(v1b[:, tb, 63:64].bitcast(F32), 1.0)

        identity = pers.tile([128, 128], F32R, tag="ident")
        nc.sync.dma_start(identity[:], ident[:])
        identity_b = pers.tile([128, 128], BF16, tag="identb")
        nc.sync.dma_start(identity_b[:], identb[:])

        xT_r = xT.rearrange("(k p) t -> k p t", p=128)

        # ================= Phase A =================
        qn_kn = {}
        for tb in range(TB):
            c, r = divmod(tb, 4)
            if r == 0:
                if c == 0:
                    x_sb = x_first
                else:
                    x_sb = [xp.tile([128, 512], F32R, tag=f"x{k}",
                                    name=f"x{k}") for k in range(KT)]
                    for k in range(KT):
                        nc.sync.dma_start(x_sb[k][:],
                                          xT_r[k, :, c * 512:(c + 1) * 512])
            qkv_ps = ps.tile([128, 1024], F32, tag="s", name="qkv_ps",
                             bufs=2)[:, 0:512]
            for k in range(KT):
                nc.tensor.matmul(qkv_ps[:, 0:386],
                                 x_sb[k][:, r * 128:(r + 1) * 128],
                                 wq_sb[k][:], start=(k == 0), stop=(k == KT - 1))
            # PSUM -> SBUF once (ACT) so rope/v-gate can run on GPSIMD
            qkv = tmp.tile([128, 386], F32, tag="qkvs")
            nc.scalar.copy(qkv[:], qkv_ps[:, 0:386])

            # gate = sigmoid(logit) via 1/(1+exp(-x)); v = qkv_v + gate*ve3
            eg = tmp.tile([128, 1], F32, tag="eg")
            nc.scalar.activation(eg[:], qkv[:, 384:385], AF.Exp, scale=-1.0)
            gp = tmp.tile([128, 1], F32, tag="gp")
            nc.vector.tensor_scalar_add(gp[:], eg[:], 1.0)
            gi = tmp.tile([128, 1], F32, tag="gi")
            nc.vector.reciprocal_approx_fast(gi[:], gp[:])
            vt = tmp.tile([128, D], F32, tag="vt")
            nc.vector.tensor_scalar_mul(vt[:], ve_sb[:, tb], gi[:])
            nc.vector.tensor_add(v1a[:, tb, 0:64], qkv[:, 320:384], vt[:])
            nc.gpsimd.tensor_copy(v1b[:, tb, 64:128], v1a[:, tb, 0:64])

            # rope: out = [x1|x1]*[c|s] + [x2|x2]*[-s|c]
            def rope(dst, src_ap, nh, eng):
                x1 = src_ap[:, :, 0:32].unsqueeze(2).broadcast_to([128, nh, 2, 32])
                x2 = src_ap[:, :, 32:64].unsqueeze(2).broadcast_to([128, nh, 2, 32])
                rav = (ra_sb[:, tb].rearrange("p (two d) -> p two d", two=2)
                       .unsqueeze(1).broadcast_to([128, nh, 2, 32]))
                rbv = (rb_sb[:, tb].rearrange("p (two d) -> p two d", two=2)
                       .unsqueeze(1).broadcast_to([128, nh, 2, 32]))
                dv = dst[:].rearrange("p (h two d) -> p h two d", h=nh, two=2)
                t1 = tmp.tile([128, nh * 64], F32, tag=f"t1{nh}")
                t1v = t1[:].rearrange("p (h two d) -> p h two d", h=nh, two=2)
                eng.tensor_tensor(t1v, x1, rav, ALU.mult)
                eng.tensor_tensor(dv, x2, rbv, ALU.mult)
                eng.tensor_add(dst[:], dst[:], t1[:])

            qr = tmp.tile([128, G * D], F32, tag="qr")
            rope(qr, qkv[:, 0:256].rearrange("p (h d) -> p h d", h=G), G,
                 nc.gpsimd)
            kr = tmp.tile([128, D], F32, tag="kr")
            rope(kr, qkv[:, 256:320].rearrange("p (h d) -> p h d", h=1), 1,
                 nc.vector)

            # rms-norm scales: rsqrt(mean(x^2)+eps) = exp(-0.5*ln(m))
            sq = tmp.tile([128, D], F32, tag="sq")
            ss = tmp.tile([128, 8], F32, tag="ss")
            for h in range(G):
                nc.scalar.activation(sq[:], qr[:, h * 64:(h + 1) * 64],
                                     AF.Square, accum_out=ss[:, h:h + 1])
            nc.scalar.activation(sq[:], kr[:], AF.Square,
                                 accum_out=ss[:, 4:5])
            m5 = tmp.tile([128, 5], F32, tag="m5")
            nc.vector.tensor_scalar(m5[:], ss[:, 0:5], 1.0 / D, RMS_EPS,
                                    ALU.mult, ALU.add)
            ln5 = tmp.tile([128, 5], F32, tag="ln5")
            nc.scalar.activation(ln5[:], m5[:], AF.Ln)
            rs5 = tmp.tile([128, 5], F32, tag="rs5")
            nc.scalar.activation(rs5[:], ln5[:], AF.Exp, scale=-0.5)

            qn = tmp.tile([128, G * D], F32R, tag="qn", bufs=4)
            for h in range(G):
                nc.vector.tensor_scalar_mul(
                    qn[:, h * 64:(h + 1) * 64], qr[:, h * 64:(h + 1) * 64],
                    rs5[:, h:h + 1])
            kn = tmp.tile([128, D], F32R, tag="kn", bufs=4)
            nc.vector.tensor_scalar_mul(kn[:], kr[:], rs5[:, 4:5])

            # transposes run 2 iterations behind so PE never waits on the
            # rope/rms chain of the current block
            qn_kn[tb] = (qn, kn)
            for dtb in ([tb - 2] if tb >= 2 else []) + \
                       ([tb - 1, tb] if tb == TB - 1 else []):
                dqn, dkn = qn_kn.pop(dtb)
                for p in range(2):
                    tq = ps.tile([128, 1024], F32R, tag=("a0", "a1")[p],
                                 name="tq", bufs=1)[:, 0:128]
                    nc.tensor.transpose(tq[:], dqn[:, p * 128:(p + 1) * 128],
                                        identity[:])
                    nc.vector.tensor_copy(
                        qt_sb[p][:, dtb * 128:(dtb + 1) * 128], tq[:])
                tk = ps.tile([128, 1024], F32R, tag="a0",
                             name="tk", bufs=1)[0:64, 0:128]
                nc.tensor.transpose(tk[:], dkn[:], identity[:])
                nc.vector.tensor_copy(kt_sb[0:64, dtb * 128:(dtb + 1) * 128],
                                      tk[:])
                nc.sync.dma_start(kt_sb[64:128, dtb * 128:(dtb + 1) * 128],
                                  kt_sb[0:64, dtb * 128:(dtb + 1) * 128])

        # ========== Phase B + C, interleaved per 512-query chunk ==========
        # Both head-pair streams advance m-by-m in lockstep so the ACT
        # engine (exp) stays saturated; out-projection for the finished
        # chunk is emitted immediately so its PSUM->SBUF copies and output
        # DMAs overlap the next chunk's attention.
        for c in range(NC_):
            ms = list(_active_m(c))
            pvs = [ps.tile([128, 1024], F32, tag=("a0", "a1")[hp],
                           name="pv", bufs=1) for hp in range(2)]
            # order blocks so a full-span m comes first: its PV matmul
            # (start=True) initializes the whole accumulator, letting every
            # later PV run trimmed to its active span without memsets.
            spans = {}
            for m in ms:
                deltas = [4 * c + qpos - m for qpos in range(4)]
                act_q = [q for q in range(4) if 0 <= deltas[q] <= 8]
                spans[m] = (act_q[0], act_q[-1] + 1, deltas)
            mf = next(m for m in ms if spans[m][0] == 0 and spans[m][1] == 4)
            ms_o = [mf] + [m for m in ms if m != mf]
            pending = {}   # hp -> (p2, mi) awaiting its PV matmuls
            for mi in range(len(ms_o) + 1):
                for hp in range(2):
                    if mi < len(ms_o):
                        m = ms_o[mi]
                        qs, qe, deltas = spans[m]
                        sqs, sqe = qs, qe
                        if sqe - sqs == 1:           # N=128 runs at 1/4 rate;
                            if sqs >= 1:             # widen to 256 (even, fast)
                                sqs -= 1
                            else:
                                sqe += 1
                        w = (sqe - sqs) * 128
                        s2 = ps.tile([128, 1024], F32, tag="s", name="s2",
                                     bufs=2)
                        for hl in range(2):
                            o = hl * 512 + sqs * 128
                            nc.tensor.matmul(
                                s2[:, o:o + w],
                                kt_sb[hl * 64:(hl + 1) * 64,
                                      m * 128:(m + 1) * 128],
                                qt_sb[hp][hl * 64:(hl + 1) * 64,
                                          c * 512 + sqs * 128:
                                          c * 512 + sqe * 128],
                                start=True, stop=False,
                                tile_position=(hl * 64, 0),
                                skip_group_check=True)
                            for qpos in range(qs, qe):
                                mt = (mc_sb if deltas[qpos] == 0 else
                                      mw_sb if deltas[qpos] == 8 else None)
                                if mt is None:
                                    continue
                                qo = hl * 512 + qpos * 128
                                nc.tensor.matmul(
                                    s2[:, qo:qo + 128], identity_b[:], mt[:],
                                    start=False, stop=False,
                                    skip_group_check=True)
                        p2 = p2p.tile([128, 1024], F32R)
                        p2v = p2[:].rearrange("p (h f) -> p h f", h=2)
                        s2v = s2[:].rearrange("p (h f) -> p h f", h=2)
                        nc.scalar.activation(
                            p2v[:, :, qs * 128:qe * 128],
                            s2v[:, :, qs * 128:qe * 128],
                            AF.Exp, scale=0.125)
                    if mi > 0:
                        prev_p2, pmi = pending[hp]
                        pm = ms_o[pmi]
                        pqs, pqe, _ = spans[pm]
                        st = (pmi == 0)
                        sp_ = (pmi == len(ms_o) - 1)
                        if st:
                            pqs, pqe = 0, 4
                        pw = (pqe - pqs) * 128
                        for half in range(2):
                            o = half * 512 + pqs * 128
                            nc.tensor.matmul(
                                pvs[hp][:, o:o + pw],
                                (v1a, v1b)[half][:, pm],
                                prev_p2[:, o:o + pw],
                                start=st, stop=sp_, skip_group_check=True)
                    if mi < len(ms_o):
                        pending[hp] = (p2, mi)
            for hp in range(2):
                pv = pvs[hp]
                # denominators: reciprocal straight from PSUM rows 63/64,
                # then partition-broadcast via DMA
                ri = tmp.tile([128, 1024], F32, tag="ri")
                nc.vector.reciprocal_approx_fast(ri[:], pv[:, :])
                rb2 = outs.tile([128, 512], F32, tag="rb2")
                nc.sync.dma_start(
                    rb2[0:64, :],
                    ri[64:65, 0:512].unsqueeze(1).broadcast_to([1, 64, 512]))
                nc.sync.dma_start(
                    rb2[64:128, :],
                    ri[63:64, 512:1024].unsqueeze(1).broadcast_to([1, 64, 512]))
                nc.vector.tensor_tensor(
                    aot[hp][0:64, c * 512:(c + 1) * 512],
                    pv[0:64, 0:512], rb2[0:64, :], ALU.mult)
                nc.vector.tensor_tensor(
                    aot[hp][64:128, c * 512:(c + 1) * 512],
                    pv[64:128, 512:1024], rb2[64:128, :], ALU.mult)
            # out-projection for this chunk; stores batched 4 t-blocks
            # per DMA to cut HWDGE serialization
            for fc in range(2):
                ob4 = outs.tile([128, 4, 512], F32, tag=f"ob{fc}",
                                name=f"ob{fc}", bufs=2)
                for r in range(4):
                    tb = 4 * c + r
                    op = ps.tile([128, 1024], F32, tag=("a1", "a0")[fc],
                                 name="op", bufs=1)[:, 0:512]
                    for k in range(2):
                        nc.tensor.matmul(op[:],
                                         aot[k][:, tb * 128:(tb + 1) * 128],
                                         wo_sb[k][:, fc * 512:(fc + 1) * 512],
                                         start=(k == 0), stop=(k == 1))
                    nc.vector.tensor_copy(ob4[:, r], op[:])
                nc.sync.dma_start(
                    out.rearrange("(cc r p) e -> cc r p e", r=4, p=128)
                       [c, :, :, fc * 512:(fc + 1) * 512]
                       .transpose([1, 0, 2]),
                    ob4[:])

    nc.compile()
    return nc


def _prep_inputs(x, value_embeds, rope_cos, rope_sin, w_qkv, w_gate, w_o):
    cos = np.asarray(rope_cos, np.float32)
    sin = np.asarray(rope_sin, np.float32)
    ropeA = np.concatenate([cos, sin], axis=1)
    ropeB = np.concatenate([-sin, cos], axis=1)
    ii = np.arange(128)[:, None]
    jj = np.arange(128)[None, :]
    import ml_dtypes
    maskC = np.where(ii <= jj, 0.0, -1e30).astype(ml_dtypes.bfloat16)
    maskW = np.where(ii >= jj, 0.0, -1e30).astype(ml_dtypes.bfloat16)
    maps = []
    for core in range(8):
        b, g = divmod(core, 4)
        wq = w_qkv[g * G * D:(g + 1) * G * D]              # [256, E]
        wk = w_qkv[(HQ + g) * D:(HQ + g + 1) * D]          # [64, E]
        wv = w_qkv[(HQ + HK + g) * D:(HQ + HK + g + 1) * D]
        gate_col = np.zeros((2, E), np.float32)
        gate_col[0, :GATE_CH] = w_gate[g]
        wqkvT = np.ascontiguousarray(
            np.concatenate([wq, wk, wv, gate_col], axis=0).T)  # [E, 386]
        maps.append({
            "xT": np.ascontiguousarray(x[b].T),
            "wqkvT": wqkvT,
            "ve3": np.ascontiguousarray(
                3.0 * value_embeds[b, :, g * D:(g + 1) * D]),
            "ropeA": ropeA, "ropeB": ropeB,
            "woT": np.ascontiguousarray(w_o[:, g * G * D:(g + 1) * G * D].T),
            "maskC": maskC, "maskW": maskW,
            "ident": np.eye(128, dtype=np.float32),
            "identb": np.eye(128, dtype=ml_dtypes.bfloat16),
        })
    return maps


def kernel(x, value_embeds, rope_cos, rope_sin, w_qkv, w_gate, w_o,
           trace=False):
    if "nc" not in _CACHE:
        _CACHE["nc"] = build_program()
    nc = _CACHE["nc"]
    in_maps = _prep_inputs(x, value_embeds, rope_cos, rope_sin,
                           w_qkv, w_gate, w_o)
    res = run_bass_kernel_spmd(nc, in_maps, list(range(8)), trace=trace)
    _CACHE["last_exec_time_ns"] = res.exec_time_ns
    out = np.empty((B, T, E), np.float32)
    for b in range(B):
        out[b] = sum(res.results[4 * b + g]["out"] for g in range(4))
    return out


# revision 28
# speedup vs baseline: 508.0585x; 1.0037x over previous
"""Trainium2 Bass kernel for nn_CausalSelfAttention_15178414424258.

GQA sliding-window causal attention (HQ=16, HK=4, D=64, WINDOW=1024) with
value-embedding gating, rope + qk rms-norm, out-projection.

Sharding: tensor-parallel over the 4 kv-head groups x data-parallel over the
2 batches = 8 cores. Each core handles one batch b and one kv group g
(4 q heads, 1 k head, 1 v head), produces a partial out-projection
(its 256 channels of the attention output against the matching w_o columns);
the host sums the 4 partials per batch.

On-core dataflow (all matmuls fp32r; scores kept transposed [t_k, t_q] so
softmax denominators come free via a ones-column appended to V):
  A) qkv = x @ w_qkv_shard^T (+ gate logit col), gate/sigmoid, v += gate*ve,
     rope via [x1|x1]*[c|s] + [x2|x2]*[-s|c], rms-norm via Ln/Exp rsqrt,
     PE-transpose q/k to [d, t].
  B) per 512-query chunk and head-pair: S^T = k^T q in PSUM, exp on ACT
     (scale=1/8 folds the 1/sqrt(D)), mask/zero edge quarters on GPSIMD,
     PV accumulate with ones-augmented V giving aoT + denom row,
     reciprocal + partition-broadcast-DMA + normalize into aoT.
  C) out_partial = aoT^T @ w_oT_shard, DMA to DRAM.
"""
import sys

sys.path.insert(0, "/opt/trn_rl_repo")

from contextlib import ExitStack  # noqa: E402

import numpy as np  # noqa: E402

import concourse.bass as bass  # noqa: E402
import concourse.tile as tile  # noqa: E402
from concourse import bacc, mybir  # noqa: E402
from concourse.bass_utils import run_bass_kernel_spmd  # noqa: E402

F32 = mybir.dt.float32
F32R = mybir.dt.float32r
BF16 = mybir.dt.bfloat16
AF = mybir.ActivationFunctionType
ALU = mybir.AluOpType
AX = mybir.AxisListType

B, T, E = 2, 2048, 1024
HQ, HK, D = 16, 4, 64
WINDOW = 1024
GATE_CH = 12
RMS_EPS = 1e-8
G = HQ // HK          # 4 q heads per kv group
TB = T // 128         # 16 t-blocks
NC_ = 4               # 512-wide query chunks
KT = E // 128         # 8 k-tiles for the qkv matmul

_CACHE = {}


def _active_m(c):
    return range(max(0, 4 * c - 8), 4 * c + 4)


def _pin_act_tables(nc):
    """Keep Exp/Ln only in the combined set so insert_act_table_loads
    emits a single table load instead of thrashing between sets."""
    from concourse import hw_specs
    tabs = hw_specs.get_activation_tables(nc.m.arch)
    for name, s in tabs.items():
        if name != "natural_log_exp_and_others":
            s.discard(AF.Exp)
            s.discard(AF.Ln)


def build_program():
    nc = bacc.Bacc("TRN2", target_bir_lowering=False, debug=False, num_devices=8)
    _pin_act_tables(nc)

    xT = nc.declare_dram_parameter("xT", [E, T], F32R, isOutput=False)
    wqkvT = nc.declare_dram_parameter("wqkvT", [E, 386], F32R, isOutput=False)
    ve3 = nc.declare_dram_parameter("ve3", [T, D], F32, isOutput=False)
    ropeA = nc.declare_dram_parameter("ropeA", [T, D], F32, isOutput=False)
    ropeB = nc.declare_dram_parameter("ropeB", [T, D], F32, isOutput=False)
    woT = nc.declare_dram_parameter("woT", [G * D, E], F32R, isOutput=False)
    maskC = nc.declare_dram_parameter("maskC", [128, 128], BF16, isOutput=False)
    maskW = nc.declare_dram_parameter("maskW", [128, 128], BF16, isOutput=False)
    ident = nc.declare_dram_parameter("ident", [128, 128], F32R, isOutput=False)
    identb = nc.declare_dram_parameter("identb", [128, 128], BF16, isOutput=False)
    out = nc.declare_dram_parameter("out", [T, E], F32, isOutput=True)

    with tile.TileContext(nc) as tc, ExitStack() as ctx:
        P = lambda **kw: ctx.enter_context(tc.tile_pool(**kw))
        pers = P(name="pers", bufs=1)
        xp = P(name="xp", bufs=2)
        tmp = P(name="tmp", bufs=2)
        p2p = P(name="p2p", bufs=4)
        outs = P(name="outs", bufs=3)
        # PSUM budget (8 banks): tag "s" 2x[128,1024] (scores + qkv/outproj),
        # tags "a0"/"a1" 1x[128,1024] each (PV accumulators, hp-alternating,
        # reused for the phase-A transposes)
        ps = P(name="ps", bufs=1, space="PSUM")

        # ---- persistent SBUF ----
        wq_sb = [pers.tile([128, 386], F32R, tag=f"wq{k}", name=f"wq{k}") for k in range(KT)]
        wo_sb = [pers.tile([128, E], F32R, tag=f"wo{k}", name=f"wo{k}") for k in range(2)]
        ra_sb = pers.tile([128, TB, D], F32, tag="ra")
        rb_sb = pers.tile([128, TB, D], F32, tag="rb")
        ve_sb = pers.tile([128, TB, D], F32, tag="ve")
        mc_sb = pers.tile([128, 128], BF16, tag="mc")
        mw_sb = pers.tile([128, 128], BF16, tag="mw")
        v1a = pers.tile([128, TB, 128], F32R, tag="v1a")   # [v | 1 | 0...]
        v1b = pers.tile([128, TB, 128], F32R, tag="v1b")   # [0...| 1 | v]
        qt_sb = [pers.tile([128, T], F32R, tag=f"qt{p}", name=f"qt{p}") for p in range(2)]
        kt_sb = pers.tile([128, T], F32R, tag="kt")  # kT duplicated in both halves
        aot = [pers.tile([128, T], F32R, tag=f"aot{p}", name=f"aot{p}") for p in range(2)]

        wq_r = wqkvT.rearrange("(k p) f -> k p f", p=128)
        wo_r = woT.rearrange("(k p) f -> k p f", p=128)
        xT_r0 = xT.rearrange("(k p) t -> k p t", p=128)
        x_first = [xp.tile([128, 512], F32R, tag=f"x{k}", name=f"x{k}")
                   for k in range(KT)]
        for k in range(KT):
            nc.sync.dma_start(x_first[k][:], xT_r0[k, :, 0:512])
            nc.sync.dma_start(wq_sb[k][:], wq_r[k])
        nc.sync.dma_start(ra_sb[:], ropeA.rearrange("(tb p) d -> p tb d", p=128))
        nc.sync.dma_start(rb_sb[:], ropeB.rearrange("(tb p) d -> p tb d", p=128))
        nc.sync.dma_start(ve_sb[:], ve3.rearrange("(tb p) d -> p tb d", p=128))
        for k in range(2):
            nc.sync.dma_start(wo_sb[k][:], wo_r[k])
        nc.sync.dma_start(mc_sb[:], maskC[:])
        nc.sync.dma_start(mw_sb[:], maskW[:])

        # ones/zeros pattern of the augmented V copies
        nc.vector.memset(v1a[:].bitcast(F32), 0.0)
        nc.vector.memset(v1b[:].bitcast(F32), 0.0)
        for tb in range(TB):
            nc.vector.memset(v1a[:, tb, 64:65].bitcast(F32), 1.0)
            nc.vector.memset(v1b[:, tb, 63:64].bitcast(F32), 1.0)

        identity = pers.tile([128, 128], F32R, tag="ident")
        nc.sync.dma_start(identity[:], ident[:])
        identity_b = pers.tile([128, 128], BF16, tag="identb")
        nc.sync.dma_start(identity_b[:], identb[:])

        xT_r = xT.rearrange("(k p) t -> k p t", p=128)

        # ================= Phase A =================
        qn_kn = {}
        for tb in range(TB):
            c, r = divmod(tb, 4)
            if r == 0:
                if c == 0:
                    x_sb = x_first
                else:
                    x_sb = [xp.tile([128, 512], F32R, tag=f"x{k}",
                                    name=f"x{k}") for k in range(KT)]
                    for k in range(KT):
                        nc.sync.dma_start(x_sb[k][:],
                                          xT_r[k, :, c * 512:(c + 1) * 512])
            qkv_ps = ps.tile([128, 1024], F32, tag="s", name="qkv_ps",
                             bufs=2)[:, 0:512]
            for k in range(KT):
                nc.tensor.matmul(qkv_ps[:, 0:386],
                                 x_sb[k][:, r * 128:(r + 1) * 128],
                                 wq_sb[k][:], start=(k == 0), stop=(k == KT - 1))
            # PSUM -> SBUF once (ACT) so rope/v-gate can run on GPSIMD
            qkv = tmp.tile([128, 386], F32, tag="qkvs")
            nc.scalar.copy(qkv[:], qkv_ps[:, 0:386])

            # gate = sigmoid(logit) via 1/(1+exp(-x)); v = qkv_v + gate*ve3
            eg = tmp.tile([128, 1], F32, tag="eg")
            nc.scalar.activation(eg[:], qkv[:, 384:385], AF.Exp, scale=-1.0)
            gp = tmp.tile([128, 1], F32, tag="gp")
            nc.vector.tensor_scalar_add(gp[:], eg[:], 1.0)
            gi = tmp.tile([128, 1], F32, tag="gi")
            nc.vector.reciprocal_approx_fast(gi[:], gp[:])
            vt = tmp.tile([128, D], F32, tag="vt")
            nc.vector.tensor_scalar_mul(vt[:], ve_sb[:, tb], gi[:])
            nc.vector.tensor_add(v1a[:, tb, 0:64], qkv[:, 320:384], vt[:])
            nc.gpsimd.tensor_copy(v1b[:, tb, 64:128], v1a[:, tb, 0:64])

            # rope: out = [x1|x1]*[c|s] + [x2|x2]*[-s|c]
            def rope(dst, src_ap, nh, eng):
                x1 = src_ap[:, :, 0:32].unsqueeze(2).broadcast_to([128, nh, 2, 32])
                x2 = src_ap[:, :, 32:64].unsqueeze(2).broadcast_to([128, nh, 2, 32])
                rav = (ra_sb[:, tb].rearrange("p (two d) -> p two d", two=2)
                       .unsqueeze(1).broadcast_to([128, nh, 2, 32]))
                rbv = (rb_sb[:, tb].rearrange("p (two d) -> p two d", two=2)
                       .unsqueeze(1).broadcast_to([128, nh, 2, 32]))
                dv = dst[:].rearrange("p (h two d) -> p h two d", h=nh, two=2)
                t1 = tmp.tile([128, nh * 64], F32, tag=f"t1{nh}")
                t1v = t1[:].rearrange("p (h two d) -> p h two d", h=nh, two=2)
                eng.tensor_tensor(t1v, x1, rav, ALU.mult)
                eng.tensor_tensor(dv, x2, rbv, ALU.mult)
                eng.tensor_add(dst[:], dst[:], t1[:])

            qr = tmp.tile([128, G * D], F32, tag="qr")
            rope(qr, qkv[:, 0:256].rearrange("p (h d) -> p h d", h=G), G,
                 nc.gpsimd)
            kr = tmp.tile([128, D], F32, tag="kr")
            rope(kr, qkv[:, 256:320].rearrange("p (h d) -> p h d", h=1), 1,
                 nc.vector)

            # rms-norm scales: rsqrt(mean(x^2)+eps) = exp(-0.5*ln(m))
            sq = tmp.tile([128, D], F32, tag="sq")
            ss = tmp.tile([128, 8], F32, tag="ss")
            for h in range(G):
                nc.scalar.activation(sq[:], qr[:, h * 64:(h + 1) * 64],
                                     AF.Square, accum_out=ss[:, h:h + 1])
            nc.scalar.activation(sq[:], kr[:], AF.Square,
                                 accum_out=ss[:, 4:5])
            m5 = tmp.tile([128, 5], F32, tag="m5")
            nc.vector.tensor_scalar(m5[:], ss[:, 0:5], 1.0 / D, RMS_EPS,
                                    ALU.mult, ALU.add)
            ln5 = tmp.tile([128, 5], F32, tag="ln5")
            nc.scalar.activation(ln5[:], m5[:], AF.Ln)
            rs5 = tmp.tile([128, 5], F32, tag="rs5")
            nc.scalar.activation(rs5[:], ln5[:], AF.Exp, scale=-0.5)

            qn = tmp.tile([128, G * D], F32R, tag="qn", bufs=4)
            for h in range(G):
                nc.vector.tensor_scalar_mul(
                    qn[:, h * 64:(h + 1) * 64], qr[:, h * 64:(h + 1) * 64],
                    rs5[:, h:h + 1])
            kn = tmp.tile([128, D], F32R, tag="kn", bufs=4)
            nc.vector.tensor_scalar_mul(kn[:], kr[:], rs5[:, 4:5])

            # transposes run 2 iterations behind so PE never waits on the
            # rope/rms chain of the current block
            qn_kn[tb] = (qn, kn)
            for dtb in ([tb - 2] if tb >= 2 else []) + \
                       ([tb - 1, tb] if tb == TB - 1 else []):
                dqn, dkn = qn_kn.pop(dtb)
                for p in range(2):
                    tq = ps.tile([128, 1024], F32R, tag=("a0", "a1")[p],
                                 name="tq", bufs=1)[:, 0:128]
                    nc.tensor.transpose(tq[:], dqn[:, p * 128:(p + 1) * 128],
                                        identity[:])
                    nc.vector.tensor_copy(
                        qt_sb[p][:, dtb * 128:(dtb + 1) * 128], tq[:])
                tk = ps.tile([128, 1024], F32R, tag="a0",
                             name="tk", bufs=1)[0:64, 0:128]
                nc.tensor.transpose(tk[:], dkn[:], identity[:])
                nc.vector.tensor_copy(kt_sb[0:64, dtb * 128:(dtb + 1) * 128],
                                      tk[:])
                if dtb % 4 == 3:
                    nc.sync.dma_start(
                        kt_sb[64:128, (dtb - 3) * 128:(dtb + 1) * 128],
                        kt_sb[0:64, (dtb - 3) * 128:(dtb + 1) * 128])

        # ========== Phase B + C, interleaved per 512-query chunk ==========
        # Both head-pair streams advance m-by-m in lockstep so the ACT
        # engine (exp) stays saturated; out-projection for the finished
        # chunk is emitted immediately so its PSUM->SBUF copies and output
        # DMAs overlap the next chunk's attention.
        for c in range(NC_):
            ms = list(_active_m(c))
            pvs = [ps.tile([128, 1024], F32, tag=("a0", "a1")[hp],
                           name="pv", bufs=1) for hp in range(2)]
            # order blocks so a full-span m comes first: its PV matmul
            # (start=True) initializes the whole accumulator, letting every
            # later PV run trimmed to its active span without memsets.
            spans = {}
            for m in ms:
                deltas = [4 * c + qpos - m for qpos in range(4)]
                act_q = [q for q in range(4) if 0 <= deltas[q] <= 8]
                spans[m] = (act_q[0], act_q[-1] + 1, deltas)
            mf = next(m for m in ms if spans[m][0] == 0 and spans[m][1] == 4)
            ms_o = [mf] + [m for m in ms if m != mf]
            pending = {}   # hp -> (p2, mi) awaiting its PV matmuls
            for mi in range(len(ms_o) + 1):
                for hp in range(2):
                    if mi < len(ms_o):
                        m = ms_o[mi]
                        qs, qe, deltas = spans[m]
                        sqs, sqe = qs, qe
                        if sqe - sqs == 1:           # N=128 runs at 1/4 rate;
                            if sqs >= 1:             # widen to 256 (even, fast)
                                sqs -= 1
                            else:
                                sqe += 1
                        w = (sqe - sqs) * 128
                        s2 = ps.tile([128, 1024], F32, tag="s", name="s2",
                                     bufs=2)
                        for hl in range(2):
                            o = hl * 512 + sqs * 128
                            nc.tensor.matmul(
                                s2[:, o:o + w],
                                kt_sb[hl * 64:(hl + 1) * 64,
                                      m * 128:(m + 1) * 128],
                                qt_sb[hp][hl * 64:(hl + 1) * 64,
                                          c * 512 + sqs * 128:
                                          c * 512 + sqe * 128],
                                start=True, stop=False,
                                tile_position=(hl * 64, 0),
                                skip_group_check=True)
                            for qpos in range(qs, qe):
                                mt = (mc_sb if deltas[qpos] == 0 else
                                      mw_sb if deltas[qpos] == 8 else None)
                                if mt is None:
                                    continue
                                qo = hl * 512 + qpos * 128
                                nc.tensor.matmul(
                                    s2[:, qo:qo + 128], identity_b[:], mt[:],
                                    start=False, stop=False,
                                    skip_group_check=True)
                        p2 = p2p.tile([128, 1024], F32R)
                        p2v = p2[:].rearrange("p (h f) -> p h f", h=2)
                        s2v = s2[:].rearrange("p (h f) -> p h f", h=2)
                        nc.scalar.activation(
                            p2v[:, :, qs * 128:qe * 128],
                            s2v[:, :, qs * 128:qe * 128],
                            AF.Exp, scale=0.125)
                    if mi > 0:
                        prev_p2, pmi = pending[hp]
                        pm = ms_o[pmi]
                        pqs, pqe, _ = spans[pm]
                        st = (pmi == 0)
                        sp_ = (pmi == len(ms_o) - 1)
                        if st:
                            pqs, pqe = 0, 4
                        pw = (pqe - pqs) * 128
                        for half in range(2):
                            o = half * 512 + pqs * 128
                            nc.tensor.matmul(
                                pvs[hp][:, o:o + pw],
                                (v1a, v1b)[half][:, pm],
                                prev_p2[:, o:o + pw],
                                start=st, stop=sp_, skip_group_check=True)
                    if mi < len(ms_o):
                        pending[hp] = (p2, mi)
            for hp in range(2):
                pv = pvs[hp]
                # denominators: reciprocal straight from PSUM rows 63/64,
                # then partition-broadcast via DMA
                ri = tmp.tile([128, 1024], F32, tag="ri")
                nc.vector.reciprocal_approx_fast(ri[:], pv[:, :])
                rb2 = outs.tile([128, 512], F32, tag="rb2")
                nc.sync.dma_start(
                    rb2[0:64, :],
                    ri[64:65, 0:512].unsqueeze(1).broadcast_to([1, 64, 512]))
                nc.sync.dma_start(
                    rb2[64:128, :],
                    ri[63:64, 512:1024].unsqueeze(1).broadcast_to([1, 64, 512]))
                nc.vector.tensor_tensor(
                    aot[hp][0:64, c * 512:(c + 1) * 512],
                    pv[0:64, 0:512], rb2[0:64, :], ALU.mult)
                nc.vector.tensor_tensor(
                    aot[hp][64:128, c * 512:(c + 1) * 512],
                    pv[64:128, 512:1024], rb2[64:128, :], ALU.mult)
            # out-projection for this chunk; stores batched 4 t-blocks
            # per DMA to cut HWDGE serialization
            for fc in range(2):
                ob4 = outs.tile([128, 4, 512], F32, tag=f"ob{fc}",
                                name=f"ob{fc}", bufs=2)
                for r in range(4):
                    tb = 4 * c + r
                    op = ps.tile([128, 1024], F32, tag=("a1", "a0")[fc],
                                 name="op", bufs=1)[:, 0:512]
                    for k in range(2):
                        nc.tensor.matmul(op[:],
                                         aot[k][:, tb * 128:(tb + 1) * 128],
                                         wo_sb[k][:, fc * 512:(fc + 1) * 512],
                                         start=(k == 0), stop=(k == 1))
                    nc.vector.tensor_copy(ob4[:, r], op[:])
                nc.sync.dma_start(
                    out.rearrange("(cc r p) e -> cc r p e", r=4, p=128)
                       [c, :, :, fc * 512:(fc + 1) * 512]
                       .transpose([1, 0, 2]),
                    ob4[:])

    nc.compile()
    return nc


def _prep_inputs(x, value_embeds, rope_cos, rope_sin, w_qkv, w_gate, w_o):
    cos = np.asarray(rope_cos, np.float32)
    sin = np.asarray(rope_sin, np.float32)
    ropeA = np.concatenate([cos, sin], axis=1)
    ropeB = np.concatenate([-sin, cos], axis=1)
    ii = np.arange(128)[:, None]
    jj = np.arange(128)[None, :]
    import ml_dtypes
    maskC = np.where(ii <= jj, 0.0, -1e30).astype(ml_dtypes.bfloat16)
    maskW = np.where(ii >= jj, 0.0, -1e30).astype(ml_dtypes.bfloat16)
    maps = []
    for core in range(8):
        b, g = divmod(core, 4)
        wq = w_qkv[g * G * D:(g + 1) * G * D]              # [256, E]
        wk = w_qkv[(HQ + g) * D:(HQ + g + 1) * D]          # [64, E]
        wv = w_qkv[(HQ + HK + g) * D:(HQ + HK + g + 1) * D]
        gate_col = np.zeros((2, E), np.float32)
        gate_col[0, :GATE_CH] = w_gate[g]
        wqkvT = np.ascontiguousarray(
            np.concatenate([wq, wk, wv, gate_col], axis=0).T)  # [E, 386]
        maps.append({
            "xT": np.ascontiguousarray(x[b].T),
            "wqkvT": wqkvT,
            "ve3": np.ascontiguousarray(
                3.0 * value_embeds[b, :, g * D:(g + 1) * D]),
            "ropeA": ropeA, "ropeB": ropeB,
            "woT": np.ascontiguousarray(w_o[:, g * G * D:(g + 1) * G * D].T),
            "maskC": maskC, "maskW": maskW,
            "ident": np.eye(128, dtype=np.float32),
            "identb": np.eye(128, dtype=ml_dtypes.bfloat16),
        })
    return maps


def kernel(x, value_embeds, rope_cos, rope_sin, w_qkv, w_gate, w_o,
           trace=False):
    if "nc" not in _CACHE:
        _CACHE["nc"] = build_program()
    nc = _CACHE["nc"]
    in_maps = _prep_inputs(x, value_embeds, rope_cos, rope_sin,
                           w_qkv, w_gate, w_o)
    res = run_bass_kernel_spmd(nc, in_maps, list(range(8)), trace=trace)
    _CACHE["last_exec_time_ns"] = res.exec_time_ns
    out = np.empty((B, T, E), np.float32)
    for b in range(B):
        out[b] = sum(res.results[4 * b + g]["out"] for g in range(4))
    return out


# revision 29
# speedup vs baseline: 521.8128x; 1.0271x over previous
"""Trainium2 Bass kernel for nn_CausalSelfAttention_15178414424258.

GQA sliding-window causal attention (HQ=16, HK=4, D=64, WINDOW=1024) with
value-embedding gating, rope + qk rms-norm, out-projection.

Sharding: tensor-parallel over the 4 kv-head groups x data-parallel over the
2 batches = 8 cores. Each core handles one batch b and one kv group g
(4 q heads, 1 k head, 1 v head), produces a partial out-projection
(its 256 channels of the attention output against the matching w_o columns);
the host sums the 4 partials per batch.

On-core dataflow (all matmuls fp32r; scores kept transposed [t_k, t_q] so
softmax denominators come free via a ones-column appended to V):
  A) qkv = x @ w_qkv_shard^T (+ gate logit col), gate/sigmoid, v += gate*ve,
     rope via [x1|x1]*[c|s] + [x2|x2]*[-s|c], rms-norm via Ln/Exp rsqrt,
     PE-transpose q/k to [d, t].
  B) per 512-query chunk and head-pair: S^T = k^T q in PSUM, exp on ACT
     (scale=1/8 folds the 1/sqrt(D)), mask/zero edge quarters on GPSIMD,
     PV accumulate with ones-augmented V giving aoT + denom row,
     reciprocal + partition-broadcast-DMA + normalize into aoT.
  C) out_partial = aoT^T @ w_oT_shard, DMA to DRAM.
"""
import sys

sys.path.insert(0, "/opt/trn_rl_repo")

from contextlib import ExitStack  # noqa: E402

import numpy as np  # noqa: E402

import concourse.bass as bass  # noqa: E402
import concourse.tile as tile  # noqa: E402
from concourse import bacc, mybir  # noqa: E402
from concourse.bass_utils import run_bass_kernel_spmd  # noqa: E402

F32 = mybir.dt.float32
F32R = mybir.dt.float32r
BF16 = mybir.dt.bfloat16
AF = mybir.ActivationFunctionType
ALU = mybir.AluOpType
AX = mybir.AxisListType

B, T, E = 2, 2048, 1024
HQ, HK, D = 16, 4, 64
WINDOW = 1024
GATE_CH = 12
RMS_EPS = 1e-8
G = HQ // HK          # 4 q heads per kv group
TB = T // 128         # 16 t-blocks
NC_ = 4               # 512-wide query chunks
KT = E // 128         # 8 k-tiles for the qkv matmul

_CACHE = {}


def _active_m(c):
    return range(max(0, 4 * c - 8), 4 * c + 4)


def _pin_act_tables(nc):
    """Keep Exp/Ln only in the combined set so insert_act_table_loads
    emits a single table load instead of thrashing between sets."""
    from concourse import hw_specs
    tabs = hw_specs.get_activation_tables(nc.m.arch)
    for name, s in tabs.items():
        if name != "natural_log_exp_and_others":
            s.discard(AF.Exp)
            s.discard(AF.Ln)


def build_program():
    nc = bacc.Bacc("TRN2", target_bir_lowering=False, debug=False, num_devices=8)
    _pin_act_tables(nc)

    xT = nc.declare_dram_parameter("xT", [E, T], F32R, isOutput=False)
    wqkvT = nc.declare_dram_parameter("wqkvT", [E, 386], F32R, isOutput=False)
    ve3 = nc.declare_dram_parameter("ve3", [T, D], F32, isOutput=False)
    ropeA = nc.declare_dram_parameter("ropeA", [T, D], F32, isOutput=False)
    ropeB = nc.declare_dram_parameter("ropeB", [T, D], F32, isOutput=False)
    woT = nc.declare_dram_parameter("woT", [G * D, E], F32R, isOutput=False)
    maskC = nc.declare_dram_parameter("maskC", [128, 128], BF16, isOutput=False)
    maskW = nc.declare_dram_parameter("maskW", [128, 128], BF16, isOutput=False)
    ident = nc.declare_dram_parameter("ident", [128, 128], F32R, isOutput=False)
    identb = nc.declare_dram_parameter("identb", [128, 128], BF16, isOutput=False)
    out = nc.declare_dram_parameter("out", [T, E], F32, isOutput=True)

    with tile.TileContext(nc) as tc, ExitStack() as ctx:
        P = lambda **kw: ctx.enter_context(tc.tile_pool(**kw))
        pers = P(name="pers", bufs=1)
        xp = P(name="xp", bufs=2)
        tmp = P(name="tmp", bufs=2)
        p2p = P(name="p2p", bufs=6)
        outs = P(name="outs", bufs=3)
        # PSUM budget (8 banks): tag "s" 2x[128,1024] (scores + qkv/outproj),
        # tags "a0"/"a1" 1x[128,1024] each (PV accumulators, hp-alternating,
        # reused for the phase-A transposes)
        ps = P(name="ps", bufs=1, space="PSUM")

        # ---- persistent SBUF ----
        wq_sb = [pers.tile([128, 386], F32R, tag=f"wq{k}", name=f"wq{k}") for k in range(KT)]
        wo_sb = [pers.tile([128, E], F32R, tag=f"wo{k}", name=f"wo{k}") for k in range(2)]
        ra_sb = pers.tile([128, TB, D], F32, tag="ra")
        rb_sb = pers.tile([128, TB, D], F32, tag="rb")
        ve_sb = pers.tile([128, TB, D], F32, tag="ve")
        mc_sb = pers.tile([128, 128], BF16, tag="mc")
        mw_sb = pers.tile([128, 128], BF16, tag="mw")
        v1a = pers.tile([128, TB, 128], F32R, tag="v1a")   # [v | 1 | 0...]
        v1b = pers.tile([128, TB, 128], F32R, tag="v1b")   # [0...| 1 | v]
        qt_sb = [pers.tile([128, T], F32R, tag=f"qt{p}", name=f"qt{p}") for p in range(2)]
        kt_sb = pers.tile([128, T], F32R, tag="kt")  # kT duplicated in both halves
        aot = [pers.tile([128, T], F32R, tag=f"aot{p}", name=f"aot{p}") for p in range(2)]

        wq_r = wqkvT.rearrange("(k p) f -> k p f", p=128)
        wo_r = woT.rearrange("(k p) f -> k p f", p=128)
        xT_r0 = xT.rearrange("(k p) t -> k p t", p=128)
        x_first = [xp.tile([128, 512], F32R, tag=f"x{k}", name=f"x{k}")
                   for k in range(KT)]
        for k in range(KT):
            nc.sync.dma_start(x_first[k][:], xT_r0[k, :, 0:512])
            nc.sync.dma_start(wq_sb[k][:], wq_r[k])
        nc.sync.dma_start(ra_sb[:], ropeA.rearrange("(tb p) d -> p tb d", p=128))
        nc.sync.dma_start(rb_sb[:], ropeB.rearrange("(tb p) d -> p tb d", p=128))
        nc.sync.dma_start(ve_sb[:], ve3.rearrange("(tb p) d -> p tb d", p=128))
        for k in range(2):
            nc.sync.dma_start(wo_sb[k][:], wo_r[k])
        nc.sync.dma_start(mc_sb[:], maskC[:])
        nc.sync.dma_start(mw_sb[:], maskW[:])

        # ones/zeros pattern of the augmented V copies
        nc.vector.memset(v1a[:].bitcast(F32), 0.0)
        nc.vector.memset(v1b[:].bitcast(F32), 0.0)
        for tb in range(TB):
            nc.vector.memset(v1a[:, tb, 64:65].bitcast(F32), 1.0)
            nc.vector.memset(v1b[:, tb, 63:64].bitcast(F32), 1.0)

        identity = pers.tile([128, 128], F32R, tag="ident")
        nc.sync.dma_start(identity[:], ident[:])
        identity_b = pers.tile([128, 128], BF16, tag="identb")
        nc.sync.dma_start(identity_b[:], identb[:])

        xT_r = xT.rearrange("(k p) t -> k p t", p=128)

        # ================= Phase A =================
        qn_kn = {}
        for tb in range(TB):
            c, r = divmod(tb, 4)
            if r == 0:
                if c == 0:
                    x_sb = x_first
                else:
                    x_sb = [xp.tile([128, 512], F32R, tag=f"x{k}",
                                    name=f"x{k}") for k in range(KT)]
                    for k in range(KT):
                        nc.sync.dma_start(x_sb[k][:],
                                          xT_r[k, :, c * 512:(c + 1) * 512])
            qkv_ps = ps.tile([128, 1024], F32, tag="s", name="qkv_ps",
                             bufs=2)[:, 0:512]
            for k in range(KT):
                nc.tensor.matmul(qkv_ps[:, 0:386],
                                 x_sb[k][:, r * 128:(r + 1) * 128],
                                 wq_sb[k][:], start=(k == 0), stop=(k == KT - 1))
            # PSUM -> SBUF once (ACT) so rope/v-gate can run on GPSIMD
            qkv = tmp.tile([128, 386], F32, tag="qkvs", bufs=3)
            nc.scalar.copy(qkv[:], qkv_ps[:, 0:386])

            # gate = sigmoid(logit) via 1/(1+exp(-x)); v = qkv_v + gate*ve3
            eg = tmp.tile([128, 1], F32, tag="eg")
            nc.scalar.activation(eg[:], qkv[:, 384:385], AF.Exp, scale=-1.0)
            gp = tmp.tile([128, 1], F32, tag="gp")
            nc.vector.tensor_scalar_add(gp[:], eg[:], 1.0)
            gi = tmp.tile([128, 1], F32, tag="gi")
            nc.vector.reciprocal_approx_fast(gi[:], gp[:])
            vt = tmp.tile([128, D], F32, tag="vt")
            nc.vector.tensor_scalar_mul(vt[:], ve_sb[:, tb], gi[:])
            nc.vector.tensor_add(v1a[:, tb, 0:64], qkv[:, 320:384], vt[:])
            nc.gpsimd.tensor_copy(v1b[:, tb, 64:128], v1a[:, tb, 0:64])

            # rope: out = [x1|x1]*[c|s] + [x2|x2]*[-s|c]
            def rope(dst, src_ap, nh, eng):
                x1 = src_ap[:, :, 0:32].unsqueeze(2).broadcast_to([128, nh, 2, 32])
                x2 = src_ap[:, :, 32:64].unsqueeze(2).broadcast_to([128, nh, 2, 32])
                rav = (ra_sb[:, tb].rearrange("p (two d) -> p two d", two=2)
                       .unsqueeze(1).broadcast_to([128, nh, 2, 32]))
                rbv = (rb_sb[:, tb].rearrange("p (two d) -> p two d", two=2)
                       .unsqueeze(1).broadcast_to([128, nh, 2, 32]))
                dv = dst[:].rearrange("p (h two d) -> p h two d", h=nh, two=2)
                t1 = tmp.tile([128, nh * 64], F32, tag=f"t1{nh}")
                t1v = t1[:].rearrange("p (h two d) -> p h two d", h=nh, two=2)
                eng.tensor_tensor(t1v, x1, rav, ALU.mult)
                eng.tensor_tensor(dv, x2, rbv, ALU.mult)
                eng.tensor_add(dst[:], dst[:], t1[:])

            qr = tmp.tile([128, G * D], F32, tag="qr")
            rope(qr, qkv[:, 0:256].rearrange("p (h d) -> p h d", h=G), G,
                 nc.gpsimd)
            kr = tmp.tile([128, D], F32, tag="kr")
            rope(kr, qkv[:, 256:320].rearrange("p (h d) -> p h d", h=1), 1,
                 nc.vector)

            # rms-norm scales: rsqrt(mean(x^2)+eps) = exp(-0.5*ln(m))
            sq = tmp.tile([128, D], F32, tag="sq")
            ss = tmp.tile([128, 8], F32, tag="ss")
            for h in range(G):
                nc.scalar.activation(sq[:], qr[:, h * 64:(h + 1) * 64],
                                     AF.Square, accum_out=ss[:, h:h + 1])
            nc.scalar.activation(sq[:], kr[:], AF.Square,
                                 accum_out=ss[:, 4:5])
            m5 = tmp.tile([128, 5], F32, tag="m5")
            nc.vector.tensor_scalar(m5[:], ss[:, 0:5], 1.0 / D, RMS_EPS,
                                    ALU.mult, ALU.add)
            ln5 = tmp.tile([128, 5], F32, tag="ln5")
            nc.scalar.activation(ln5[:], m5[:], AF.Ln)
            rs5 = tmp.tile([128, 5], F32, tag="rs5")
            nc.scalar.activation(rs5[:], ln5[:], AF.Exp, scale=-0.5)

            qn = tmp.tile([128, G * D], F32R, tag="qn", bufs=4)
            for h in range(G):
                nc.vector.tensor_scalar_mul(
                    qn[:, h * 64:(h + 1) * 64], qr[:, h * 64:(h + 1) * 64],
                    rs5[:, h:h + 1])
            kn = tmp.tile([128, D], F32R, tag="kn", bufs=4)
            nc.vector.tensor_scalar_mul(kn[:], kr[:], rs5[:, 4:5])

            # transposes run 2 iterations behind so PE never waits on the
            # rope/rms chain of the current block
            qn_kn[tb] = (qn, kn)
            for dtb in ([tb - 2] if tb >= 2 else []) + \
                       ([tb - 1, tb] if tb == TB - 1 else []):
                dqn, dkn = qn_kn.pop(dtb)
                for p in range(2):
                    tq = ps.tile([128, 1024], F32R, tag=("a0", "a1")[p],
                                 name="tq", bufs=1)[:, 0:128]
                    nc.tensor.transpose(tq[:], dqn[:, p * 128:(p + 1) * 128],
                                        identity[:])
                    nc.vector.tensor_copy(
                        qt_sb[p][:, dtb * 128:(dtb + 1) * 128], tq[:])
                tk = ps.tile([128, 1024], F32R, tag="a0",
                             name="tk", bufs=1)[0:64, 0:128]
                nc.tensor.transpose(tk[:], dkn[:], identity[:])
                nc.vector.tensor_copy(kt_sb[0:64, dtb * 128:(dtb + 1) * 128],
                                      tk[:])
                if dtb % 4 == 3:
                    nc.sync.dma_start(
                        kt_sb[64:128, (dtb - 3) * 128:(dtb + 1) * 128],
                        kt_sb[0:64, (dtb - 3) * 128:(dtb + 1) * 128])

        # ========== Phase B + C, interleaved per 512-query chunk ==========
        # Both head-pair streams advance m-by-m in lockstep so the ACT
        # engine (exp) stays saturated; out-projection for the finished
        # chunk is emitted immediately so its PSUM->SBUF copies and output
        # DMAs overlap the next chunk's attention.
        for c in range(NC_):
            ms = list(_active_m(c))
            pvs = [ps.tile([128, 1024], F32, tag=("a0", "a1")[hp],
                           name="pv", bufs=1) for hp in range(2)]
            # order blocks so a full-span m comes first: its PV matmul
            # (start=True) initializes the whole accumulator, letting every
            # later PV run trimmed to its active span without memsets.
            spans = {}
            for m in ms:
                deltas = [4 * c + qpos - m for qpos in range(4)]
                act_q = [q for q in range(4) if 0 <= deltas[q] <= 8]
                spans[m] = (act_q[0], act_q[-1] + 1, deltas)
            mf = next(m for m in ms if spans[m][0] == 0 and spans[m][1] == 4)
            ms_o = [mf] + [m for m in ms if m != mf]
            pending = {}   # hp -> (p2, mi) awaiting its PV matmuls
            for mi in range(len(ms_o) + 1):
                for hp in range(2):
                    if mi < len(ms_o):
                        m = ms_o[mi]
                        qs, qe, deltas = spans[m]
                        sqs, sqe = qs, qe
                        if sqe - sqs == 1:           # N=128 runs at 1/4 rate;
                            if sqs >= 1:             # widen to 256 (even, fast)
                                sqs -= 1
                            else:
                                sqe += 1
                        w = (sqe - sqs) * 128
                        s2 = ps.tile([128, 1024], F32, tag="s", name="s2",
                                     bufs=2)
                        for hl in range(2):
                            o = hl * 512 + sqs * 128
                            nc.tensor.matmul(
                                s2[:, o:o + w],
                                kt_sb[hl * 64:(hl + 1) * 64,
                                      m * 128:(m + 1) * 128],
                                qt_sb[hp][hl * 64:(hl + 1) * 64,
                                          c * 512 + sqs * 128:
                                          c * 512 + sqe * 128],
                                start=True, stop=False,
                                tile_position=(hl * 64, 0),
                                skip_group_check=True)
                            for qpos in range(qs, qe):
                                mt = (mc_sb if deltas[qpos] == 0 else
                                      mw_sb if deltas[qpos] == 8 else None)
                                if mt is None:
                                    continue
                                qo = hl * 512 + qpos * 128
                                nc.tensor.matmul(
                                    s2[:, qo:qo + 128], identity_b[:], mt[:],
                                    start=False, stop=False,
                                    skip_group_check=True)
                        p2 = p2p.tile([128, 1024], F32R)
                        p2v = p2[:].rearrange("p (h f) -> p h f", h=2)
                        s2v = s2[:].rearrange("p (h f) -> p h f", h=2)
                        nc.scalar.activation(
                            p2v[:, :, qs * 128:qe * 128],
                            s2v[:, :, qs * 128:qe * 128],
                            AF.Exp, scale=0.125)
                    if mi > 0:
                        prev_p2, pmi = pending[hp]
                        pm = ms_o[pmi]
                        pqs, pqe, _ = spans[pm]
                        st = (pmi == 0)
                        sp_ = (pmi == len(ms_o) - 1)
                        if st:
                            pqs, pqe = 0, 4
                        pw = (pqe - pqs) * 128
                        for half in range(2):
                            o = half * 512 + pqs * 128
                            nc.tensor.matmul(
                                pvs[hp][:, o:o + pw],
                                (v1a, v1b)[half][:, pm],
                                prev_p2[:, o:o + pw],
                                start=st, stop=sp_, skip_group_check=True)
                    if mi < len(ms_o):
                        pending[hp] = (p2, mi)
            for hp in range(2):
                pv = pvs[hp]
                # denominators: reciprocal straight from PSUM rows 63/64,
                # then partition-broadcast via DMA
                ri = tmp.tile([128, 1024], F32, tag="ri")
                nc.vector.reciprocal_approx_fast(ri[:], pv[:, :])
                rb2 = outs.tile([128, 512], F32, tag="rb2")
                nc.sync.dma_start(
                    rb2[0:64, :],
                    ri[64:65, 0:512].unsqueeze(1).broadcast_to([1, 64, 512]))
                nc.sync.dma_start(
                    rb2[64:128, :],
                    ri[63:64, 512:1024].unsqueeze(1).broadcast_to([1, 64, 512]))
                nc.vector.tensor_tensor(
                    aot[hp][0:64, c * 512:(c + 1) * 512],
                    pv[0:64, 0:512], rb2[0:64, :], ALU.mult)
                nc.vector.tensor_tensor(
                    aot[hp][64:128, c * 512:(c + 1) * 512],
                    pv[64:128, 512:1024], rb2[64:128, :], ALU.mult)
            # out-projection for this chunk; stores batched 4 t-blocks
            # per DMA to cut HWDGE serialization
            for fc in range(2):
                ob4 = outs.tile([128, 4, 512], F32, tag=f"ob{fc}",
                                name=f"ob{fc}", bufs=2)
                for r in range(4):
                    tb = 4 * c + r
                    op = ps.tile([128, 1024], F32, tag=("a1", "a0")[fc],
                                 name="op", bufs=1)[:, 0:512]
                    for k in range(2):
                        nc.tensor.matmul(op[:],
                                         aot[k][:, tb * 128:(tb + 1) * 128],
                                         wo_sb[k][:, fc * 512:(fc + 1) * 512],
                                         start=(k == 0), stop=(k == 1))
                    nc.vector.tensor_copy(ob4[:, r], op[:])
                nc.sync.dma_start(
                    out.rearrange("(cc r p) e -> cc r p e", r=4, p=128)
                       [c, :, :, fc * 512:(fc + 1) * 512]
                       .transpose([1, 0, 2]),
                    ob4[:])

    nc.compile()
    return nc


def _prep_inputs(x, value_embeds, rope_cos, rope_sin, w_qkv, w_gate, w_o):
    cos = np.asarray(rope_cos, np.float32)
    sin = np.asarray(rope_sin, np.float32)
    ropeA = np.concatenate([cos, sin], axis=1)
    ropeB = np.concatenate([-sin, cos], axis=1)
    ii = np.arange(128)[:, None]
    jj = np.arange(128)[None, :]
    import ml_dtypes
    maskC = np.where(ii <= jj, 0.0, -1e30).astype(ml_dtypes.bfloat16)
    maskW = np.where(ii >= jj, 0.0, -1e30).astype(ml_dtypes.bfloat16)
    maps = []
    for core in range(8):
        b, g = divmod(core, 4)
        wq = w_qkv[g * G * D:(g + 1) * G * D]              # [256, E]
        wk = w_qkv[(HQ + g) * D:(HQ + g + 1) * D]          # [64, E]
        wv = w_qkv[(HQ + HK + g) * D:(HQ + HK + g + 1) * D]
        gate_col = np.zeros((2, E), np.float32)
        gate_col[0, :GATE_CH] = w_gate[g]
        wqkvT = np.ascontiguousarray(
            np.concatenate([wq, wk, wv, gate_col], axis=0).T)  # [E, 386]
        maps.append({
            "xT": np.ascontiguousarray(x[b].T),
            "wqkvT": wqkvT,
            "ve3": np.ascontiguousarray(
                3.0 * value_embeds[b, :, g * D:(g + 1) * D]),
            "ropeA": ropeA, "ropeB": ropeB,
            "woT": np.ascontiguousarray(w_o[:, g * G * D:(g + 1) * G * D].T),
            "maskC": maskC, "maskW": maskW,
            "ident": np.eye(128, dtype=np.float32),
            "identb": np.eye(128, dtype=ml_dtypes.bfloat16),
        })
    return maps


def kernel(x, value_embeds, rope_cos, rope_sin, w_qkv, w_gate, w_o,
           trace=False):
    if "nc" not in _CACHE:
        _CACHE["nc"] = build_program()
    nc = _CACHE["nc"]
    in_maps = _prep_inputs(x, value_embeds, rope_cos, rope_sin,
                           w_qkv, w_gate, w_o)
    res = run_bass_kernel_spmd(nc, in_maps, list(range(8)), trace=trace)
    _CACHE["last_exec_time_ns"] = res.exec_time_ns
    out = np.empty((B, T, E), np.float32)
    for b in range(B):
        out[b] = sum(res.results[4 * b + g]["out"] for g in range(4))
    return out
